# revision 1
# baseline (speedup 1.0000x reference)
"""Trainium2 Bass kernel for nn_DirectedODRLoss (retrieval_knn).

Math (B=4096, D=256, k=25, scales (1,2,3)):
    Inputs are sorted by score on the host (the loss is invariant under a
    global permutation of sample indices).  With sorted scores,
    pen_ij = relu(s_i - s_j) = (s_i - s_j) * [j < i]  (strict lower tri), so
        V := pen @ P^T,  V[i,k] = s_i * C_k(i) - D_k(i)
    with C_k(i) = sum_{j<i} P[k,j], D_k(i) = sum_{j<i} s_j P[k,j] — plain
    exclusive prefix sums along sorted columns, computed in O(B^2) with
    per-128-chunk triangular matmuls + rank-1 carry matmuls.  This removes
    one of the two B^3 GEMMs and the P all-gather of the previous version.

    dist²(i,j) = |f_i|² + |f_j|² − 2 f_i·f_j ;  y := −dist²
    topk:  per row, 25 largest y via DVE max8 + match_replace;
        τ_i := 25th largest;  σ_i = mean(sqrt(−y_clamped+eps))
    mutual knn:  y symmetric ⇒ mutual(i,j) = [y_ij ≥ max(τ_i, τ_j)]
    dir(i,j) = [lab_i ≤ lab_j] folded into the threshold: +BIG when violated
    Wn = exp(y·rσ_i·rσ_j)·keep,  S_i = ΣWn + 1,  P = Wn/S + diag(1/S)
    loss = (1/B)(C1 + C2/2 + C3/3),
        C1 = <P, pen>,  C2 = <P, V>,  C3 = <A, V>,  A = P @ P[:,M_c]
    column-sharded: core c holds A[:, M_c], V[:, M_c] (V from OWN rows only).

Sharding: rows split across 8 cores in sorted order.  Only P^T is
all-gathered (bf16, 32 MB); A's lhsT comes from ptfull slices and its rhs
P[:, M_c] from XBAR DMA transposes of ptfull.  y/W strips are fp16 in SBUF;
wide [128, 4096] DVE/scalar instructions amortize instruction overhead.
"""

import ml_dtypes
import numpy as np

import concourse.bacc as bacc
import concourse.bass as bass
import concourse.mybir as mybir
import concourse.tile as tile

F32 = mybir.dt.float32
F16 = mybir.dt.float16
BF16 = mybir.dt.bfloat16
AX = mybir.AxisListType
OP = mybir.AluOpType
ACT = mybir.ActivationFunctionType

EPS = 1e-8
KNN = 25
BIG = 30000.0
NEG_INF = -60000.0


def build_program(B=4096, D=256, NC=8):
    P = 128
    R = B // NC            # rows per core (512)
    NMT = R // P           # row tiles per core (4)
    KC = B // P            # 128-row chunks of B (32)
    GK = D // P            # contraction chunks for the Gram GEMM (2)
    NW = B // 512          # 512-wide column chunks (8)

    nc = bacc.Bacc("TRN2", target_bir_lowering=False, debug=False,
                   num_devices=NC)

    # ---- I/O ----------------------------------------------------------------
    ftf = nc.dram_tensor("ftf", [D, B], BF16, kind="ExternalInput")    # Fᵀ full
    ft2 = nc.dram_tensor("ft2", [D, R], BF16, kind="ExternalInput")    # 2·F_shᵀ
    fsh = nc.dram_tensor("fsh", [R, D], F32, kind="ExternalInput")     # F shard
    srow16 = nc.dram_tensor("srow16", [1, B], F16, kind="ExternalInput")
    lrow16 = nc.dram_tensor("lrow16", [1, B], F16, kind="ExternalInput")
    scolsf = nc.dram_tensor("scolsf", [P, KC], F32, kind="ExternalInput")
    sc_own = nc.dram_tensor("sc_own", [P, NMT], F32, kind="ExternalInput")
    labBIG = nc.dram_tensor("labBIG", [P, NMT], F32, kind="ExternalInput")
    # host-built constants
    tri_in = nc.dram_tensor("tri_in", [P, P], BF16, kind="ExternalInput")
    tribc_in = nc.dram_tensor("tribc_in", [KC, KC * P], BF16,
                              kind="ExternalInput")
    sel_in = nc.dram_tensor("sel_in", [P, KC * KC], BF16, kind="ExternalInput")
    loss_out = nc.dram_tensor("loss", [1, 1], F32, kind="ExternalOutput")

    # ---- internal DRAM ------------------------------------------------------
    pt_dram = nc.dram_tensor("pt_dram", [B, R], BF16)                  # P_shᵀ
    H = B // 2
    ptfull_a = nc.dram_tensor("ptfull_a", [NC * H, R], BF16, addr_space="Shared")
    ptfull_b = nc.dram_tensor("ptfull_b", [NC * H, R], BF16, addr_space="Shared")
    a2a_dram = nc.dram_tensor("a2a_dram", [B, R], BF16)
    v_dram = nc.dram_tensor("v_dram", [B, R], BF16)
    invs_dram = nc.dram_tensor("invs_dram", [P, NMT], F32)
    stats_in = nc.dram_tensor("stats_in", [1, 2 * R], F32)
    stats_out = nc.dram_tensor("stats_out", [NC, 2 * R], F32, addr_space="Shared")
    red_in = nc.dram_tensor("red_in", [1, 8], F32)
    red_out = nc.dram_tensor("red_out", [1, 8], F32, addr_space="Shared")

    rg = [list(range(NC))]

    with tile.TileContext(nc) as tc:
        with (
            tc.tile_pool(name="const", bufs=1) as constp,
            tc.tile_pool(name="big", bufs=1) as bigp,
            tc.tile_pool(name="cols", bufs=1) as colp,
            tc.tile_pool(name="work", bufs=1) as workp,
            tc.tile_pool(name="sw", bufs=2) as swp,
            tc.tile_pool(name="deep", bufs=6) as deepp,
            tc.tile_pool(name="psum", bufs=1, space="PSUM") as psump,
        ):
            def ps_tile(tag, shape=None, dtype=F32):
                return psump.tile(shape or [P, 512], dtype, tag=tag, name=tag)

            # ============ consts ============================================
            tri128 = constp.tile([P, P], BF16, tag="tri128")  # [p<m] strict
            nc.sync.dma_start(tri128[:], tri_in[:, :])
            # tribc[:, t*128:(t+1)*128] = column t of strict-upper TRI32
            # replicated 128x: carry-add lhsT (sums cs rows u<t inside the mm)
            tribc = constp.tile([KC, KC * P], BF16, tag="tribc")
            nc.sync.dma_start(tribc[:], tribc_in[:, :])
            # sel_u [128, KC] with column u all-ones: colsum of chunk u lands
            # on psum partition u when used as matmul lhsT (accumulated)
            sel = constp.tile([P, KC * KC], BF16, tag="sel")
            nc.sync.dma_start(sel[:], sel_in[:, :])
            ones1f = constp.tile([1, P], F32, tag="ones1f")
            nc.vector.memset(ones1f[:], 1.0)
            ones128f = constp.tile([P, 1], F32, tag="ones128f")
            nc.vector.memset(ones128f[:], 1.0)
            eps_c = constp.tile([P, 1], F32, tag="eps_c")
            nc.vector.memset(eps_c[:], EPS)

            # ============ input loads =======================================
            ft2_sb = constp.tile([P, GK * R], BF16, tag="ft2_sb")
            for g in range(GK):
                nc.sync.dma_start(ft2_sb[:, g * R:(g + 1) * R],
                                  ft2[g * P:(g + 1) * P, :])
            # slotA: ftf (Gram) -> pn strips -> V
            ftf_sb = bigp.tile([P, GK * B], BF16, tag="A", name="ftf_sb")
            for g in range(GK):
                nc.sync.dma_start(ftf_sb[:, g * B:(g + 1) * B],
                                  ftf[g * P:(g + 1) * P, :])
            s_col = colp.tile([P, KC], F32, tag="s_col")
            nc.sync.dma_start(s_col[:], scolsf[:, :])
            s_c = colp.tile([P, NMT], F32, tag="s_c")
            nc.sync.dma_start(s_c[:], sc_own[:, :])
            labB_c = colp.tile([P, NMT], F32, tag="labB_c")
            nc.sync.dma_start(labB_c[:], labBIG[:, :])
            s_b16 = constp.tile([P, B], F16, tag="s_b16")
            nc.sync.dma_start(s_b16[:], bass.AP(srow16, 0, [[0, P], [1, B]]))
            lab_b16 = constp.tile([P, B], F16, tag="lab_b16")
            nc.sync.dma_start(lab_b16[:], bass.AP(lrow16, 0, [[0, P], [1, B]]))

            # own |f_i|² in col layout (bias for the y copy-out)
            sqcs = colp.tile([P, NMT], F32, tag="sqcs")
            for q in range(NMT):
                ftile = swp.tile([P, 512], F32, tag="sqq", name=f"fsh{q}")
                nc.sync.dma_start(ftile[:, 0:D], fsh[q * P:(q + 1) * P, :])
                scr = swp.tile([P, 512], F32, tag="st32", name=f"fsq{q}")
                nc.scalar.activation(scr[:, 0:D], ftile[:, 0:D], ACT.Square,
                                     accum_out=sqcs[:, q:q + 1])
            sqcs_neg = colp.tile([P, NMT], F32, tag="sqcs_neg")
            nc.vector.tensor_scalar(sqcs_neg[:], sqcs[:], -1.0, None, OP.mult)

            # |f_j|² row strip: square ftf chunks, partition-reduce by matmul,
            # then broadcast back across partitions by a K=1 matmul.
            sqb16 = workp.tile([P, B], F16, tag="t4", name="sqb16")
            for o in range(NW):
                pso = ps_tile(f"pb{o}")
                for g in range(GK):
                    sqq = swp.tile([P, 512], F32, tag="sqq",
                                   name=f"sqq{o}_{g}")
                    nc.scalar.activation(
                        sqq[:], ftf_sb[:, g * B + o * 512: g * B + (o + 1) * 512],
                        ACT.Square)
                    nc.tensor.matmul(pso[0:1, :], ones128f[:], sqq[:],
                                     start=(g == 0), stop=(g == GK - 1))
                sqr = swp.tile([P, 512], F32, tag="st32", name=f"sqr{o}")
                nc.scalar.activation(sqr[0:1, :], pso[0:1, :], ACT.Copy)
                nc.tensor.matmul(pso[:], ones1f[:], sqr[0:1, :],
                                 start=True, stop=True)
                nc.vector.tensor_copy(sqb16[:, o * 512:(o + 1) * 512], pso[:])

            # ============ Gram -> y (fp16 strips) + topk ====================
            # slotB: y strips (-> Wn in place) -> rhs_all after
            y_all = bigp.tile([P, NMT * B], F16, tag="B", name="y_all")
            vals = colp.tile([P, NMT * 32], F16, tag="vals")
            yt_cols = colp.tile([P, NMT], F32, tag="yt_cols")
            rs_cols = colp.tile([P, NMT], F32, tag="rs_cols")
            ssum = colp.tile([P, NMT], F32, tag="ssum")
            for mt in range(NMT):
                ys = y_all[:, mt * B:(mt + 1) * B]
                for o in range(NW):
                    pso = ps_tile(f"pb{o}")
                    for g in range(GK):
                        nc.tensor.matmul(
                            pso[:],
                            ft2_sb[:, g * R + mt * P: g * R + (mt + 1) * P],
                            ftf_sb[:, g * B + o * 512: g * B + (o + 1) * 512],
                            start=(g == 0), stop=(g == GK - 1))
                    # y = 2ffT - |f_i|^2 - |f_j|^2  (one fused DVE op)
                    nc.vector.scalar_tensor_tensor(
                        ys[:, o * 512:(o + 1) * 512], pso[:],
                        sqcs_neg[:, mt:mt + 1],
                        sqb16[:, o * 512:(o + 1) * 512],
                        op0=OP.add, op1=OP.subtract)
                # top-k: 4 rounds of max8 + match_replace (t1/t2 reused by W)
                sa = workp.tile([P, B], F16, tag="t1", name=f"sa{mt}")
                sb = workp.tile([P, B], F16, tag="t2", name=f"sb{mt}")
                v = vals[:, mt * 32:(mt + 1) * 32]
                nc.vector.max(out=v[:, 0:8], in_=ys)
                nc.vector.match_replace(out=sb[:], in_to_replace=v[:, 0:8],
                                        in_values=ys, imm_value=NEG_INF)
                nc.vector.max(out=v[:, 8:16], in_=sb[:])
                nc.vector.match_replace(out=sa[:], in_to_replace=v[:, 8:16],
                                        in_values=sb[:], imm_value=NEG_INF)
                nc.vector.max(out=v[:, 16:24], in_=sa[:])
                nc.vector.match_replace(out=sb[:], in_to_replace=v[:, 16:24],
                                        in_values=sa[:], imm_value=NEG_INF)
                nc.vector.max(out=v[:, 24:32], in_=sb[:])
                # τ_i = 25th largest y
                nc.vector.tensor_copy(yt_cols[:, mt:mt + 1], v[:, 24:25])
                # σ_i = mean sqrt(max(d,0)+eps) over 25 NN;  d = −y
                c25 = swp.tile([P, KNN], F32, tag="c25")
                nc.vector.tensor_scalar(c25[:], v[:, 0:KNN], 0.0, None, OP.min)
                s25 = swp.tile([P, KNN], F32, tag="s25")
                nc.scalar.activation(s25[:], c25[:], ACT.Sqrt,
                                     bias=eps_c[:, 0:1], scale=-1.0,
                                     accum_out=ssum[:, mt:mt + 1])
            nc.vector.reciprocal(rs_cols[:], ssum[:])
            nc.vector.tensor_scalar(rs_cols[:], rs_cols[:], float(KNN), None,
                                    OP.mult)

            # stats all-gather: flat per-rank [τ(R) ++ rσ(R)], shard-row order
            nc.sync.dma_start(bass.AP(stats_in, 0, [[1, P], [P, NMT]]),
                              yt_cols[:])
            nc.sync.dma_start(bass.AP(stats_in, R, [[1, P], [P, NMT]]),
                              rs_cols[:])
            nc.gpsimd.collective_compute(
                "AllGather", OP.bypass, replica_groups=rg,
                ins=[stats_in.ap().opt()], outs=[stats_out.ap().opt()])

            # broadcast τ/rσ rows, converting to fp16 in 512-wide chunks
            yt_b16 = workp.tile([P, B], F16, tag="t4", name="yt_b16")
            rs_b16 = workp.tile([P, B], F16, tag="t5", name="rs_b16")
            for rr in range(NC):
                for off, dst in ((0, yt_b16), (R, rs_b16)):
                    st32 = swp.tile([P, R], F32, tag="st32",
                                    name=f"st32_{rr}_{off}")
                    nc.sync.dma_start(
                        st32[:],
                        bass.AP(stats_out, rr * 2 * R + off, [[0, P], [1, R]]))
                    nc.vector.tensor_copy(dst[:, rr * R:(rr + 1) * R], st32[:])

            # ============ stage W: Wn, S, C1 ================================
            S_col = colp.tile([P, NMT], F32, tag="S_col")
            c1cols = colp.tile([P, NMT], F32, tag="c1cols")
            invS = colp.tile([P, NMT], F32, tag="invS")
            Scol = colp.tile([P, NMT], F32, tag="Scol")
            # slotA reuse: pn strips replace ftf
            pn_all = bigp.tile([P, NMT * B], BF16, tag="A", name="pn_all")
            # slotC: Pnᵀ[:, M_c] assembled from per-strip XBAR transposes
            ptSx = bigp.tile([P, KC * R], BF16, tag="C", name="ptSx")
            for mt in range(NMT):
                ys = y_all[:, mt * B:(mt + 1) * B]
                # direction violated -> +4000 on the threshold (fp16-safe:
                # y ∈ [-2500, 0], thr ≥ -900, so thr2 > 0 ≥ y kills the pair)
                ind = workp.tile([P, B], F16, tag="t1", name=f"ind{mt}")
                nc.vector.tensor_scalar(ind[:], lab_b16[:],
                                        labB_c[:, mt:mt + 1], None, OP.is_lt)
                thr = workp.tile([P, B], F16, tag="t2", name=f"thr{mt}")
                nc.vector.tensor_scalar(thr[:], yt_b16[:],
                                        yt_cols[:, mt:mt + 1], None, OP.max)
                thr2 = workp.tile([P, B], F16, tag="t6", name=f"thr2{mt}")
                nc.vector.scalar_tensor_tensor(
                    thr2[:], ind[:], 4000.0, thr[:], op0=OP.mult, op1=OP.add)
                keep = workp.tile([P, B], F16, tag="t2", name=f"keep{mt}")
                nc.vector.tensor_tensor(keep[:], ys, thr2[:], OP.is_ge)
                # full exp argument in one fused op: (y·rσ_i)·rσ_j
                e = workp.tile([P, B], F16, tag="t1", name=f"e{mt}")
                nc.vector.scalar_tensor_tensor(
                    e[:], ys, rs_cols[:, mt:mt + 1], rs_b16[:],
                    op0=OP.mult, op1=OP.mult)
                w0 = workp.tile([P, B], F16, tag="t6", name=f"w0{mt}")
                nc.scalar.activation(w0[:], e[:], ACT.Exp)
                # Wn = w0·keep (into the y strip), accumulating S = ΣWn
                nc.vector.scalar_tensor_tensor(
                    ys, w0[:], 1.0, keep[:], op0=OP.mult, op1=OP.mult,
                    accum_out=S_col[:, mt:mt + 1])
                # C1 partial: Σ Wn·pen
                pen = workp.tile([P, B], F16, tag="t2", name=f"pen{mt}")
                nc.scalar.activation(pen[:], s_b16[:], ACT.Relu,
                                     bias=s_c[:, mt:mt + 1], scale=-1.0)
                junk = workp.tile([P, B], F16, tag="t1", name=f"cj{mt}")
                nc.vector.scalar_tensor_tensor(
                    junk[:], ys, 1.0, pen[:], op0=OP.mult, op1=OP.mult,
                    accum_out=c1cols[:, mt:mt + 1])
                # per-strip S -> invS -> Pn scale (scalar) -> XBAR transposes,
                # all pipelined under the next strip's W ops
                nc.vector.tensor_scalar(Scol[:, mt:mt + 1],
                                        S_col[:, mt:mt + 1], 1.0, None, OP.add)
                nc.vector.reciprocal(invS[:, mt:mt + 1], Scol[:, mt:mt + 1])
                pns = pn_all[:, mt * B:(mt + 1) * B]
                nc.scalar.activation(pns, ys, ACT.Copy,
                                     scale=invS[:, mt:mt + 1])
                for t in range(KC):
                    eng = nc.scalar if t % 2 == 1 else nc.sync
                    eng.dma_start_transpose(
                        ptSx[:, t * R + mt * P: t * R + (mt + 1) * P],
                        pn_all[:, mt * B + t * P: mt * B + (t + 1) * P])

            for t in range(KC):
                eng = nc.sync if t % 2 == 0 else nc.scalar
                eng.dma_start(pt_dram[t * P:(t + 1) * P, :],
                              ptSx[:, t * R:(t + 1) * R])

            # ============ diagonal fix: P += diag(1/S) (in pt_dram) =========
            # pt flat index of (j = c*R+k, k) is c*R*R + k*(R+1)
            nc.sync.dma_start(invs_dram[:, :], invS[:])
            invs_rowf = colp.tile([1, R], F32, tag="invs_rowf")
            nc.sync.dma_start(invs_rowf[:].rearrange("a (c p) -> a c p", p=P),
                              bass.AP(invs_dram, 0, [[0, 1], [1, NMT], [NMT, P]]))
            invs_row = colp.tile([1, R], BF16, tag="invs_row")
            nc.vector.tensor_copy(invs_row[:], invs_rowf[:])
            rank = nc.gpsimd.partition_id()
            diag_ap = pt_dram.ap().rearrange("a b -> () (a b)")[
                0:1, bass.ds(rank * R * R, R, R + 1)]
            nc.gpsimd.dma_start(diag_ap, invs_row[0:1, :], accum_op=OP.add)

            # ============ collectives: AllToAll (rhs) then AG (lhsT) ========
            # AllToAll block r = pt rows [r*R,(r+1)*R) = Pᵀ[M_r, M_c]; core c
            # receives block c' = Pᵀ[M_c, M_c'] i.e. a2a[c'*R+a, b] =
            # P[c'*R+b, c*R+a] — transposing gives P[:, M_c] rank-independently.
            # It goes first so rhs_all + the C2/cumsum pass overlap the big AG.
            nc.gpsimd.collective_compute(
                "AllToAll", OP.bypass, replica_groups=rg,
                ins=[pt_dram.ap().opt()], outs=[a2a_dram.ap().opt()])
            # AG split in j-halves: the GEMM's first 16 contraction chunks
            # only need the first half, so AG_b overlaps the GEMM
            nc.gpsimd.collective_compute(
                "AllGather", OP.bypass, replica_groups=rg,
                ins=[pt_dram[0:H, :].opt()], outs=[ptfull_a.ap().opt()])
            nc.gpsimd.collective_compute(
                "AllGather", OP.bypass, replica_groups=rg,
                ins=[pt_dram[H:B, :].opt()], outs=[ptfull_b.ap().opt()])
            # ============ V = pen·Pᵀ column shard via prefix sums ===========
            # (overlaps the AllGather: streams own chunks from pt_dram, which
            # also picks up the diagonal added above)
            # pass 1: per-chunk column sums (one-hot lhsT accumulated rows)
            csC_t = ps_tile("pb4")
            csD_t = ps_tile("pb5")
            for t in range(KC):
                ptc = swp.tile([P, R], BF16, tag="ptc", name=f"ptc1_{t}")
                nc.scalar.dma_start(ptc[:], pt_dram[t * P:(t + 1) * P, :])
                swt = swp.tile([P, 512], BF16, tag="sw1", name=f"sw1_{t}")
                nc.vector.tensor_scalar(swt[:], ptc[:],
                                        s_col[:, t:t + 1], None, OP.mult)
                nc.tensor.matmul(csC_t[0:KC, :], sel[:, t * KC:(t + 1) * KC],
                                 ptc[:],
                                 start=(t == 0), stop=(t == KC - 1))
                nc.tensor.matmul(csD_t[0:KC, :], sel[:, t * KC:(t + 1) * KC],
                                 swt[:], start=(t == 0), stop=(t == KC - 1))
            cs_sbC = colp.tile([KC, 512], BF16, tag="cs_sbC")
            cs_sbD = colp.tile([KC, 512], BF16, tag="cs_sbD")
            nc.scalar.activation(cs_sbC[:], csC_t[0:KC, :], ACT.Copy)
            nc.scalar.activation(cs_sbD[:], csD_t[0:KC, :], ACT.Copy)
            # rhs_all = P[:, M_c] from a2a (XBAR); slotB reuse after y strips
            rhs_all = bigp.tile([P, KC * R], BF16, tag="B", name="rhs_all")
            for u in range(KC):
                cp = u // NMT
                ul = (u % NMT) * P
                nc.sync.dma_start_transpose(
                    rhs_all[:, u * R:(u + 1) * R],
                    a2a_dram[cp * R:(cp + 1) * R, ul:ul + P])

            # pass 2: per-chunk exclusive prefix + carry -> V chunk [i, k].
            # The carry Σ_{u<t} cs[u] is added inside the accumulation group
            # by a K=32 matmul with lhsT = tribc (column t of strict-upper
            # TRI32 replicated across the 128 output partitions).  Each V
            # chunk is contracted against rhs_all (C2) and spilled to DRAM
            # for the transposed reload used by the C3 contraction.
            c2cols = colp.tile([P, KC], F32, tag="c2cols")
            for t in range(KC):
                ptc = swp.tile([P, R], BF16, tag="ptc", name=f"ptc2_{t}")
                nc.scalar.dma_start(ptc[:], pt_dram[t * P:(t + 1) * P, :])
                swt = swp.tile([P, 512], BF16, tag="sw1", name=f"sw2_{t}")
                nc.vector.tensor_scalar(swt[:], ptc[:],
                                        s_col[:, t:t + 1], None, OP.mult)
                cpsL = ps_tile(f"pb{(t % 2) * 2}")
                cpsR = ps_tile(f"pb{(t % 2) * 2 + 1}")
                nc.tensor.matmul(cpsL[:], tri128[:], ptc[:],
                                 start=True, stop=False)
                nc.tensor.matmul(cpsL[:], tribc[:, t * P:(t + 1) * P],
                                 cs_sbC[:], start=False, stop=True)
                nc.tensor.matmul(cpsR[:], tri128[:], swt[:],
                                 start=True, stop=False)
                nc.tensor.matmul(cpsR[:], tribc[:, t * P:(t + 1) * P],
                                 cs_sbD[:], start=False, stop=True)
                dsb = swp.tile([P, 512], F32, tag="st32", name=f"dsb{t}")
                nc.scalar.activation(dsb[:], cpsR[:], ACT.Copy)
                vch = swp.tile([P, 512], BF16, tag="vch", name=f"vch{t}")
                nc.vector.scalar_tensor_tensor(
                    vch[:], cpsL[:], s_col[:, t:t + 1], dsb[:],
                    op0=OP.mult, op1=OP.subtract)
                nc.sync.dma_start(v_dram[t * P:(t + 1) * P, :], vch[:])
                junk2 = swp.tile([P, 512], F16, tag="jk", name=f"j2_{t}")
                nc.vector.scalar_tensor_tensor(
                    junk2[:], rhs_all[:, t * R:(t + 1) * R], 1.0, vch[:],
                    op0=OP.mult, op1=OP.mult,
                    accum_out=c2cols[:, t:t + 1])

            # ============ main GEMM: Aᵀ = P[:,M_c]ᵀ @ Pᵀ + C3 ===============
            # out tile (kt, ib) [128 k, 512 i]; lhsT = rhs_all slices (SBUF),
            # moving rhs = plain ptfull rows; contract vs XBAR-transposed V
            NIB = B // R  # i-blocks of width R: block ib = ptfull block ib
            HK = KC // 2
            # contract each AG half separately (C3 is linear in the split):
            # the half-a sweep's PE work fully hides AG_b
            c3cols = colp.tile([P, 2 * NMT * NIB], F32, tag="c3cols")
            for half in range(2):
                ptf = ptfull_a if half == 0 else ptfull_b
                for ib in range(NIB):
                    pg = 4 * ((half * NIB + ib) % 2)
                    psTs = [psump.tile([P, 512], F32, tag=f"pb{kt + pg}",
                                       name=f"pT{half}_{ib}_{kt}")
                            for kt in range(NMT)]
                    vts = []
                    for kt in range(NMT):
                        vt = swp.tile([P, R], BF16, tag="vt",
                                      name=f"vt{half}_{ib}_{kt}")
                        veng = nc.sync if kt % 2 == 0 else nc.scalar
                        veng.dma_start_transpose(
                            vt[:], v_dram[ib * R:(ib + 1) * R,
                                          kt * P:(kt + 1) * P])
                        vts.append(vt)
                    for uu in range(HK):
                        u = half * HK + uu
                        ptb = deepp.tile([P, R], BF16, tag="ptb",
                                         name=f"ptb{half}_{ib}_{uu}")
                        peng = nc.scalar if uu % 2 == 0 else nc.sync
                        peng.dma_start(
                            ptb[:],
                            ptf[ib * H + uu * P: ib * H + (uu + 1) * P, :])
                        for kt in range(NMT):
                            nc.tensor.matmul(
                                psTs[kt][:],
                                rhs_all[:, u * R + kt * P: u * R + (kt + 1) * P],
                                ptb[:], start=(uu == 0), stop=(uu == HK - 1))
                    for kt in range(NMT):
                        junk3 = swp.tile([P, R], F16, tag="jk",
                                         name=f"j3_{half}_{ib}_{kt}")
                        cidx = half * NMT * NIB + ib * NMT + kt
                        nc.vector.scalar_tensor_tensor(
                            junk3[:], psTs[kt][:], 1.0, vts[kt][:],
                            op0=OP.mult, op1=OP.mult,
                            accum_out=c3cols[:, cidx:cidx + 1])

            # ============ final reduction ==================================
            c1r = colp.tile([P, NMT], F32, tag="c1r")
            nc.vector.tensor_tensor(c1r[:], c1cols[:], invS[:], OP.mult)
            c1v = colp.tile([P, 1], F32, tag="c1v")
            nc.vector.reduce_sum(c1v[:], c1r[:], axis=AX.X)
            c2v = colp.tile([P, 1], F32, tag="c2v")
            c3v = colp.tile([P, 1], F32, tag="c3v")
            nc.vector.reduce_sum(c2v[:], c2cols[:], axis=AX.X)
            nc.vector.reduce_sum(c3v[:], c3cols[:], axis=AX.X)
            tot = colp.tile([P, 1], F32, tag="tot")
            nc.vector.tensor_scalar(tot[:], c2v[:], 0.5, None, OP.mult)
            nc.vector.tensor_tensor(tot[:], tot[:], c1v[:], OP.add)
            nc.vector.tensor_scalar(c3v[:], c3v[:], 1.0 / 3.0, None, OP.mult)
            nc.vector.tensor_tensor(tot[:], tot[:], c3v[:], OP.add)

            fin = ps_tile("pb3")
            nc.tensor.matmul(fin[0:1, 0:1], tot[:], ones128f[:],
                             start=True, stop=True)
            lsb = colp.tile([1, 8], F32, tag="lsb")
            nc.vector.memset(lsb[:], 0.0)
            nc.scalar.activation(lsb[:, 0:1], fin[0:1, 0:1], ACT.Copy,
                                 scale=1.0 / float(B))
            nc.sync.dma_start(red_in[:, :], lsb[:])
            nc.gpsimd.collective_compute(
                "AllReduce", OP.add, replica_groups=rg,
                ins=[red_in.ap().opt()], outs=[red_out.ap().opt()])
            nc.sync.dma_start(loss_out[:, :], red_out[0:1, 0:1])

    nc.compile()
    return nc


def make_inputs(features, scores, labels, B, D, NC):
    """Build the per-core input maps from full inputs (sorted by score)."""
    R = B // NC
    P = 128
    NMT = R // P
    KC = B // P
    s0 = np.ascontiguousarray(scores, dtype=np.float32).reshape(B)
    order = np.argsort(s0, kind="stable")
    f = np.ascontiguousarray(np.asarray(features, dtype=np.float32)[order])
    s = s0[order]
    lab = np.asarray(labels).astype(np.float32).reshape(B)[order]
    ftf = np.ascontiguousarray(f.T)
    # device constants
    tri128 = np.triu(np.ones((P, P), dtype=np.float32), 1)  # [p<m]
    tri32 = np.triu(np.ones((KC, KC), dtype=np.float32), 1)
    tribc = np.repeat(tri32[:, :, None], P, axis=2).reshape(KC, KC * P)
    sel = np.zeros((P, KC * KC), dtype=np.float32)
    for u in range(KC):
        sel[:, u * KC + u] = 1.0
    in_maps = []
    for c in range(NC):
        sh = slice(c * R, (c + 1) * R)
        in_maps.append({
            "ftf": ftf.astype(ml_dtypes.bfloat16),
            "ft2": np.ascontiguousarray(2.0 * f[sh].T).astype(ml_dtypes.bfloat16),
            "fsh": np.ascontiguousarray(f[sh]),
            "srow16": s.reshape(1, B).astype(np.float16),
            "lrow16": lab.reshape(1, B).astype(np.float16),
            "scolsf": np.ascontiguousarray(s.reshape(KC, P).T),
            "sc_own": np.ascontiguousarray(s[sh].reshape(NMT, P).T),
            "labBIG": np.ascontiguousarray(lab[sh].reshape(NMT, P).T),
            "tri_in": tri128.astype(ml_dtypes.bfloat16),
            "tribc_in": tribc.astype(ml_dtypes.bfloat16),
            "sel_in": sel.astype(ml_dtypes.bfloat16),
        })
    return in_maps


_cached = {}


def kernel(features, scores, labels):
    B, D = features.shape
    NC = 8
    key = (B, D)
    if key not in _cached:
        _cached[key] = build_program(B=B, D=D, NC=NC)
    nc = _cached[key]
    from concourse.bass_utils import run_bass_kernel_spmd
    in_maps = make_inputs(features, scores, labels, B, D, NC)
    res = run_bass_kernel_spmd(nc, in_maps, core_ids=list(range(NC)))
    out = res.results[0]["loss"]
    return np.float32(out.reshape(())[()])



# revision 8
# speedup vs baseline: 1.2597x; 1.2597x over previous
"""Trainium2 Bass kernel for nn_DirectedODRLoss (retrieval_knn).

Math (B=4096, D=256, k=25, scales (1,2,3)), rows sorted by score on host:
    pen_ij = relu(s_i - s_j) = (s_i - s_j)[j < i]  (strict lower tri), so
        V := pen @ P^T,  V[i,k] = s_i C_k(i) - D_k(i)
    with C_k(i) = sum_{j<i} P[k,j], D_k(i) = sum_{j<i} s_j P[k,j] -- exclusive
    prefix sums along sorted columns via per-128-chunk triangular matmuls plus
    rank-1 carry matmuls (O(B^2)).

    dist2(i,j) = |f_i|^2 + |f_j|^2 - 2 f_i.f_j ;  y := -dist2 (fp16 strips)
    topk: 3 rounds of DVE max8 + match_replace (top-24), then a plain
        reduce-max of the remainder gives the 25th value tau_i exactly.
    sigma_i = mean sqrt(-y_clamped+eps) over the 25 NN;  rsig = 1/sigma
    mutual knn: y symmetric => mutual(i,j) = [y_ij >= max(tau_i, tau_j)]
    direction [lab_i <= lab_j] folded in via vb = lab4k_i - lab4k_j - 2000
        (labels pre-scaled by 4000): thr2 = max(tau_i, tau_j, vb)
    Wn = exp(y rsig_i rsig_j) keep,  S_i = sum Wn + 1,
    P = Wn/S + diag(1/S)  (diagonal realized as direct writes of 2/S)
    loss = (1/B)(C1 + C2/2 + C3/3)
        C1 = <P, pen>,  C2 = <P, V>,  C3 = <P@P, V> = <P, Q>,  Q = P^T V

Distribution (8 cores, row shard M_c = rows [c*512, (c+1)*512)):
    - stats (tau, rsig) all-gathered per 128-row strip (f16, 4 tiny AGs
      pipelined behind the next strip's topk).
    - P shard is built row-major (fp16 pn_dram + fp8 pn8_dram).  The fp8
      strips are all-gathered per strip right after each W strip completes
      (4 AGs of 0.5 MB in), giving full row-major P in fp8 for the GEMM.
    - AllToAll of pn (block r' = own rows, columns M_r') delivers
      a2a_out = P[:, M_c] row-major = the elementwise partner for C2/C3.
    - V[:, M_c] is computed from own P^T chunks (XBAR-transposed out of
      pn_dram on the fly) and stays SBUF-resident as the GEMM's rhs.
    - C3 GEMM (the only B^3 term): Q_mt[jt] = sum_{i in strip mt}
      P8[i,jt]^T @ V8[i,:], fp8e4 DoubleRow matmuls (K=256/instr);
      C3 += <Q_mt[jt], P[jt-chunk, M_c]> accumulated per psum tile, so Q is
      never materialized and the contraction overlaps the remaining AGs.
"""

import ml_dtypes
import numpy as np

import concourse.bacc as bacc
import concourse.bass as bass
import concourse.mybir as mybir
import concourse.tile as tile

F32 = mybir.dt.float32
F16 = mybir.dt.float16
BF16 = mybir.dt.bfloat16
F8 = mybir.dt.float8e4
AX = mybir.AxisListType
OP = mybir.AluOpType
ACT = mybir.ActivationFunctionType
DR = mybir.MatmulPerfMode.DoubleRow

EPS = 1e-8
KNN = 25
NEG_INF = -60000.0


def build_program(B=4096, D=256, NC=8):
    P = 128
    R = B // NC            # rows per core (512)
    NMT = R // P           # row strips per core (4)
    KC = B // P            # 128-row chunks of B (32)
    GK = D // P            # contraction chunks for the Gram GEMM (2)
    NW = B // 512          # 512-wide column chunks (8)
    HK = KC // 2           # chunks per vch8 half (16)

    nc = bacc.Bacc("TRN2", target_bir_lowering=False, debug=False,
                   num_devices=NC)

    # ---- I/O ----------------------------------------------------------------
    ftf = nc.dram_tensor("ftf", [D, B], BF16, kind="ExternalInput")    # F^T full
    ft2 = nc.dram_tensor("ft2", [D, R], BF16, kind="ExternalInput")    # 2 F_sh^T
    fsh = nc.dram_tensor("fsh", [R, D], F32, kind="ExternalInput")     # F shard
    srow16 = nc.dram_tensor("srow16", [1, B], F16, kind="ExternalInput")
    lrow16 = nc.dram_tensor("lrow16", [1, B], F16, kind="ExternalInput")  # lab*4000
    scolsf = nc.dram_tensor("scolsf", [P, KC], F32, kind="ExternalInput")
    sc_own = nc.dram_tensor("sc_own", [P, NMT], F32, kind="ExternalInput")
    labBIG = nc.dram_tensor("labBIG", [P, NMT], F32, kind="ExternalInput")  # lab*4000
    tri_in = nc.dram_tensor("tri_in", [P, P], F16, kind="ExternalInput")
    tribc_in = nc.dram_tensor("tribc_in", [KC, KC * P], F16,
                              kind="ExternalInput")
    sel_in = nc.dram_tensor("sel_in", [P, KC * KC], F16, kind="ExternalInput")
    loss_out = nc.dram_tensor("loss", [1, 1], F32, kind="ExternalOutput")
    dbg_out = nc.dram_tensor("dbg", [P, 8], F32, kind="ExternalOutput")
    vch_out = nc.dram_tensor("vch_out", [P, (B // P) * 512], F16,
                             kind="ExternalOutput")
    rhs_out = nc.dram_tensor("rhs_out", [P, (B // P) * 512], F16,
                             kind="ExternalOutput")

    # ---- internal DRAM ------------------------------------------------------
    pn_dram = nc.dram_tensor("pn_dram", [R, B], F16)       # P shard row-major
    pn8_dram = nc.dram_tensor("pn8_dram", [R, B], F8)      # fp8 copy
    pnf8 = [nc.dram_tensor(f"pnf8_{mt}", [NC * P, B], F8, addr_space="Shared")
            for mt in range(NMT)]
    pt_dram = nc.dram_tensor("pt_dram", [B, R], F16)       # P_sh^T (own)
    a2a_in = nc.dram_tensor("a2a_in", [B, R], F16)
    a2a_out = nc.dram_tensor("a2a_out", [B, R], F16)       # P[:, M_c]
    st_in = [nc.dram_tensor(f"st_in_{mt}", [1, 2 * P], F16)
             for mt in range(NMT)]
    st_out = [nc.dram_tensor(f"st_out_{mt}", [NC, 2 * P], F16,
                             addr_space="Shared") for mt in range(NMT)]
    invs_dram = nc.dram_tensor("invs_dram", [P, NMT], F32)
    red_in = nc.dram_tensor("red_in", [1, 8], F32)
    red_out = nc.dram_tensor("red_out", [1, 8], F32, addr_space="Shared")

    rg = [list(range(NC))]

    with tile.TileContext(nc) as tc:
        with (
            tc.tile_pool(name="const", bufs=1) as constp,
            tc.tile_pool(name="wide", bufs=1) as widep,
            tc.tile_pool(name="big", bufs=1) as bigp,
            tc.tile_pool(name="cols", bufs=1) as colp,
            tc.tile_pool(name="work", bufs=1) as workp,
            tc.tile_pool(name="sw", bufs=2) as swp,
            tc.tile_pool(name="psum", bufs=1, space="PSUM") as psump,
        ):
            def ps_tile(tag, shape=None, dtype=F32):
                return psump.tile(shape or [P, 512], dtype, tag=tag, name=tag)

            # ============ consts ============================================
            tri128 = constp.tile([P, P], F16, tag="tri128")  # [p<m] strict
            nc.sync.dma_start(tri128[:], tri_in[:, :])
            tribc = constp.tile([KC, KC * P], F16, tag="tribc")
            nc.sync.dma_start(tribc[:], tribc_in[:, :])
            sel = constp.tile([P, KC * KC], F16, tag="sel")
            nc.sync.dma_start(sel[:], sel_in[:, :])
            ones128f = constp.tile([P, 1], F32, tag="ones128f")
            nc.vector.memset(ones128f[:], 1.0)
            ones1f = constp.tile([1, P], F32, tag="ones1f")
            nc.vector.memset(ones1f[:], 1.0)
            eps_c = constp.tile([P, 1], F32, tag="eps_c")
            nc.vector.memset(eps_c[:], EPS)

            # ============ input loads =======================================
            ft2_sb = constp.tile([P, GK * R], BF16, tag="ft2_sb")
            for g in range(GK):
                nc.sync.dma_start(ft2_sb[:, g * R:(g + 1) * R],
                                  ft2[g * P:(g + 1) * P, :])
            # slotA: ftf (Gram) -> pnf8_sb (odd GEMM strips)
            ftf_sb = bigp.tile([P, GK * B], BF16, tag="A", name="ftf_sb")
            for g in range(GK):
                nc.sync.dma_start(ftf_sb[:, g * B:(g + 1) * B],
                                  ftf[g * P:(g + 1) * P, :])
            s_col = colp.tile([P, KC], F32, tag="s_col")
            nc.sync.dma_start(s_col[:], scolsf[:, :])
            s_c = colp.tile([P, NMT], F32, tag="s_c")
            nc.sync.dma_start(s_c[:], sc_own[:, :])
            lab4k_c = colp.tile([P, NMT], F32, tag="lab4k_c")
            nc.sync.dma_start(lab4k_c[:], labBIG[:, :])
            # lab4k_i - 2000 per own row
            labm2k = colp.tile([P, NMT], F32, tag="labm2k")
            nc.vector.tensor_scalar(labm2k[:], lab4k_c[:], 2000.0, None,
                                    OP.subtract)
            # wide broadcast rows (tags w1..w4; reused by rhs_all after W)
            s_b16 = widep.tile([P, B], F16, tag="w1", name="s_b16")
            nc.sync.dma_start(s_b16[:], bass.AP(srow16, 0, [[0, P], [1, B]]))
            lab4k_b16 = widep.tile([P, B], F16, tag="w2", name="lab4k_b16")
            nc.sync.dma_start(lab4k_b16[:], bass.AP(lrow16, 0, [[0, P], [1, B]]))
            yt_b16 = widep.tile([P, B], F16, tag="w3", name="yt_b16")
            rs_b16 = widep.tile([P, B], F16, tag="w4", name="rs_b16")

            # own |f_i|^2 in col layout (bias for the y copy-out)
            sqcs = colp.tile([P, NMT], F32, tag="sqcs")
            for q in range(NMT):
                ftile = swp.tile([P, 512], F32, tag="sqq", name=f"fsh{q}")
                nc.sync.dma_start(ftile[:, 0:D], fsh[q * P:(q + 1) * P, :])
                scr = swp.tile([P, 512], F32, tag="st32", name=f"fsq{q}")
                nc.scalar.activation(scr[:, 0:D], ftile[:, 0:D], ACT.Square,
                                     accum_out=sqcs[:, q:q + 1])
            sqcs_neg = colp.tile([P, NMT], F32, tag="sqcs_neg")
            nc.vector.tensor_scalar(sqcs_neg[:], sqcs[:], -1.0, None, OP.mult)

            # |f_j|^2 row strip (partition-reduce by matmul, then broadcast)
            sqb16 = workp.tile([P, B], F16, tag="t4", name="sqb16")
            for o in range(NW):
                pso = ps_tile(f"pb{o}")
                for g in range(GK):
                    sqq = swp.tile([P, 512], F32, tag="sqq",
                                   name=f"sqq{o}_{g}")
                    nc.scalar.activation(
                        sqq[:], ftf_sb[:, g * B + o * 512: g * B + (o + 1) * 512],
                        ACT.Square)
                    nc.tensor.matmul(pso[0:1, :], ones128f[:], sqq[:],
                                     start=(g == 0), stop=(g == GK - 1))
                sqr = swp.tile([P, 512], F32, tag="st32", name=f"sqr{o}")
                nc.scalar.activation(sqr[0:1, :], pso[0:1, :], ACT.Copy)
                nc.tensor.matmul(pso[:], ones1f[:], sqr[0:1, :],
                                 start=True, stop=True)
                nc.vector.tensor_copy(sqb16[:, o * 512:(o + 1) * 512], pso[:])

            # ============ Gram -> y (fp16 strips) + topk + stats ============
            # slotB: y strips (Wn in place) -> vch after
            y_all = bigp.tile([P, NMT * B], F16, tag="B", name="y_all")
            vals = colp.tile([P, NMT * 32], F16, tag="vals")
            yt_cols = colp.tile([P, NMT], F32, tag="yt_cols")
            rs_cols = colp.tile([P, NMT], F32, tag="rs_cols")
            ssum = colp.tile([P, NMT], F32, tag="ssum")
            stf16 = colp.tile([P, 2 * NMT], F16, tag="stf16")
            for mt in range(NMT):
                ys = y_all[:, mt * B:(mt + 1) * B]
                for o in range(NW):
                    pso = ps_tile(f"pb{o}")
                    for g in range(GK):
                        nc.tensor.matmul(
                            pso[:],
                            ft2_sb[:, g * R + mt * P: g * R + (mt + 1) * P],
                            ftf_sb[:, g * B + o * 512: g * B + (o + 1) * 512],
                            start=(g == 0), stop=(g == GK - 1))
                    nc.vector.scalar_tensor_tensor(
                        ys[:, o * 512:(o + 1) * 512], pso[:],
                        sqcs_neg[:, mt:mt + 1],
                        sqb16[:, o * 512:(o + 1) * 512],
                        op0=OP.add, op1=OP.subtract)
                # top-25: 3 rounds max8 + match_replace, then plain reduce-max
                sa = workp.tile([P, B], F16, tag="t1", name=f"sa{mt}")
                sb = workp.tile([P, B], F16, tag="t2", name=f"sb{mt}")
                v = vals[:, mt * 32:(mt + 1) * 32]
                nc.vector.max(out=v[:, 0:8], in_=ys)
                nc.vector.match_replace(out=sb[:], in_to_replace=v[:, 0:8],
                                        in_values=ys, imm_value=NEG_INF)
                nc.vector.max(out=v[:, 8:16], in_=sb[:])
                nc.vector.match_replace(out=sa[:], in_to_replace=v[:, 8:16],
                                        in_values=sb[:], imm_value=NEG_INF)
                nc.vector.max(out=v[:, 16:24], in_=sa[:])
                nc.vector.match_replace(out=sb[:], in_to_replace=v[:, 16:24],
                                        in_values=sa[:], imm_value=NEG_INF)
                nc.vector.reduce_max(v[:, 24:25], sb[:], axis=AX.X)
                nc.vector.tensor_copy(yt_cols[:, mt:mt + 1], v[:, 24:25])
                # sigma_i = mean sqrt(max(d,0)+eps) over 25 NN;  d = -y
                c25 = swp.tile([P, KNN], F32, tag="c25")
                nc.vector.tensor_scalar(c25[:], v[:, 0:KNN], 0.0, None, OP.min)
                s25 = swp.tile([P, KNN], F32, tag="s25")
                nc.scalar.activation(s25[:], c25[:], ACT.Sqrt,
                                     bias=eps_c[:, 0:1], scale=-1.0,
                                     accum_out=ssum[:, mt:mt + 1])
                nc.vector.reciprocal(rs_cols[:, mt:mt + 1], ssum[:, mt:mt + 1])
                nc.vector.tensor_scalar(rs_cols[:, mt:mt + 1],
                                        rs_cols[:, mt:mt + 1], float(KNN),
                                        None, OP.mult)
                # per-strip stats AG (f16): [tau(128) ++ rsig(128)]
                nc.vector.tensor_copy(stf16[:, 2 * mt:2 * mt + 1],
                                      yt_cols[:, mt:mt + 1])
                nc.vector.tensor_copy(stf16[:, 2 * mt + 1:2 * mt + 2],
                                      rs_cols[:, mt:mt + 1])
                nc.sync.dma_start(bass.AP(st_in[mt], 0, [[1, P], [P, 2]]),
                                  stf16[:, 2 * mt:2 * mt + 2])
                nc.gpsimd.collective_compute(
                    "AllGather", OP.bypass, replica_groups=rg,
                    ins=[st_in[mt].ap().opt()], outs=[st_out[mt].ap().opt()])
                # broadcast tau/rsig rows for this strip's columns
                for r in range(NC):
                    eng = nc.sync if r % 2 == 0 else nc.scalar
                    eng.dma_start(
                        yt_b16[:, r * R + mt * P: r * R + (mt + 1) * P],
                        bass.AP(st_out[mt], r * 2 * P, [[0, P], [1, P]]))
                    eng.dma_start(
                        rs_b16[:, r * R + mt * P: r * R + (mt + 1) * P],
                        bass.AP(st_out[mt], r * 2 * P + P, [[0, P], [1, P]]))

            # ============ stage W: Wn, S, C1, P strips + collectives ========
            S_col = colp.tile([P, NMT], F32, tag="S_col")
            c1cols = colp.tile([P, NMT], F32, tag="c1cols")
            invS = colp.tile([P, NMT], F32, tag="invS")
            Scol = colp.tile([P, NMT], F32, tag="Scol")
            rank = nc.gpsimd.partition_id()
            pn_flat = pn_dram.ap().rearrange("a b -> () (a b)")
            pn8_flat = pn8_dram.ap().rearrange("a b -> () (a b)")
            a2a_flat = a2a_in.ap().rearrange("a b -> () (a b)")
            for mt in range(NMT):
                ys = y_all[:, mt * B:(mt + 1) * B]
                # vb = lab4k_i - lab4k_j - 2000  (>=2000 iff direction violated)
                vb = workp.tile([P, B], F16, tag="t1", name=f"vb{mt}")
                nc.vector.tensor_scalar(vb[:], lab4k_b16[:], -1.0,
                                        labm2k[:, mt:mt + 1], OP.mult, OP.add)
                # thr2 = max(tau_i, tau_j, vb)
                thr2 = workp.tile([P, B], F16, tag="t2", name=f"thr2{mt}")
                nc.vector.scalar_tensor_tensor(
                    thr2[:], yt_b16[:], yt_cols[:, mt:mt + 1], vb[:],
                    op0=OP.max, op1=OP.max)
                keep = workp.tile([P, B], F16, tag="t1", name=f"keep{mt}")
                nc.vector.tensor_tensor(keep[:], ys, thr2[:], OP.is_ge)
                # full exp argument in one fused op: (y rsig_i) rsig_j
                e = workp.tile([P, B], F16, tag="t6", name=f"e{mt}")
                nc.vector.scalar_tensor_tensor(
                    e[:], ys, rs_cols[:, mt:mt + 1], rs_b16[:],
                    op0=OP.mult, op1=OP.mult)
                w0 = workp.tile([P, B], F16, tag="t2", name=f"w0{mt}")
                nc.scalar.activation(w0[:], e[:], ACT.Exp)
                # Wn = w0 keep (into the y strip), accumulating S
                nc.vector.scalar_tensor_tensor(
                    ys, w0[:], 1.0, keep[:], op0=OP.mult, op1=OP.mult,
                    accum_out=S_col[:, mt:mt + 1])
                # C1 partial: sum Wn pen
                pen = workp.tile([P, B], F16, tag="t6", name=f"pen{mt}")
                nc.scalar.activation(pen[:], s_b16[:], ACT.Relu,
                                     bias=s_c[:, mt:mt + 1], scale=-1.0)
                junk = workp.tile([P, B], F16, tag="t1", name=f"cj{mt}")
                nc.vector.scalar_tensor_tensor(
                    junk[:], ys, 1.0, pen[:], op0=OP.mult, op1=OP.mult,
                    accum_out=c1cols[:, mt:mt + 1])
                nc.vector.tensor_scalar(Scol[:, mt:mt + 1],
                                        S_col[:, mt:mt + 1], 1.0, None, OP.add)
                nc.vector.reciprocal(invS[:, mt:mt + 1], Scol[:, mt:mt + 1])
                # P strip: fp16 (pn_dram) + fp8 (pn8_dram)
                pn16 = workp.tile([P, B], F16, tag="t4", name=f"pn16_{mt}")
                nc.scalar.activation(pn16[:], ys, ACT.Copy,
                                     scale=invS[:, mt:mt + 1])
                pn8 = workp.tile([P, B], F8, tag="t5", name=f"pn8_{mt}")
                nc.vector.tensor_scalar(pn8[:], ys, invS[:, mt:mt + 1], None,
                                        OP.mult)
                nc.gpsimd.dma_start(pn_dram[mt * P:(mt + 1) * P, :], pn16[:])
                nc.gpsimd.dma_start(pn8_dram[mt * P:(mt + 1) * P, :], pn8[:])
                # AllToAll input: block r' = own strip rows, columns M_r'
                for rp in range(NC):
                    nc.gpsimd.dma_start(
                        a2a_in[rp * R + mt * P: rp * R + (mt + 1) * P, :],
                        pn16[:, rp * R:(rp + 1) * R])
                # diagonal: write 2/S (self weight Wn=1 plus eye)
                nc.sync.dma_start(invs_dram[:, mt:mt + 1], invS[:, mt:mt + 1])
                d_rowf = swp.tile([1, P], F32, tag="d_rowf", name=f"drf{mt}")
                nc.sync.dma_start(d_rowf[:],
                                  bass.AP(invs_dram, mt, [[0, 1], [NMT, P]]))
                d2_16 = swp.tile([1, P], F16, tag="d2_16", name=f"d16{mt}")
                nc.vector.tensor_scalar(d2_16[:], d_rowf[:], 2.0, None, OP.mult)
                d2_8 = swp.tile([1, P], F8, tag="d2_8", name=f"d8{mt}")
                nc.vector.tensor_scalar(d2_8[:], d_rowf[:], 2.0, None, OP.mult)
                nc.gpsimd.dma_start(
                    pn_flat[0:1, bass.ds(rank * R + mt * P * (B + 1), P, B + 1)],
                    d2_16[0:1, :])
                nc.gpsimd.dma_start(
                    pn8_flat[0:1, bass.ds(rank * R + mt * P * (B + 1), P, B + 1)],
                    d2_8[0:1, :])
                nc.gpsimd.dma_start(
                    a2a_flat[0:1,
                             bass.ds(rank * R * R + mt * P * (R + 1), P, R + 1)],
                    d2_16[0:1, :])
                # per-strip fp8 AllGather of P rows (row-major); strips 2/3
                # are issued after the A2A below so the CC queue runs
                # AG_0, AG_1, A2A, AG_2, AG_3.
                if mt < 2:
                    nc.gpsimd.collective_compute(
                        "AllGather", OP.bypass, replica_groups=rg,
                        ins=[pn8_dram[mt * P:(mt + 1) * P, :].opt()],
                        outs=[pnf8[mt].ap().opt()])

            # A2A needs all strips' a2a_in writes (program order!)
            nc.gpsimd.collective_compute(
                "AllToAll", OP.bypass, replica_groups=rg,
                ins=[a2a_in.ap().opt()], outs=[a2a_out.ap().opt()])
            for mt in (2, 3):
                nc.gpsimd.collective_compute(
                    "AllGather", OP.bypass, replica_groups=rg,
                    ins=[pn8_dram[mt * P:(mt + 1) * P, :].opt()],
                    outs=[pnf8[mt].ap().opt()])

            # ============ V pass 1: transpose chunks + column sums ==========
            csC_t = ps_tile("pb6")
            csD_t = ps_tile("pb7")
            for t in range(KC):
                engA = nc.sync if t % 2 == 0 else nc.scalar
                engB = nc.scalar if t % 2 == 0 else nc.sync
                ptc = swp.tile([P, R], F16, tag="ptc", name=f"ptc1_{t}")
                engA.dma_start_transpose(ptc[:],
                                         pn_dram[0:R, t * P:(t + 1) * P])
                swt = swp.tile([P, 512], F16, tag="sw1", name=f"sw1_{t}")
                nc.vector.tensor_scalar(swt[:], ptc[:],
                                        s_col[:, t:t + 1], None, OP.mult)
                nc.tensor.matmul(csC_t[0:KC, :], sel[:, t * KC:(t + 1) * KC],
                                 ptc[:], start=(t == 0), stop=(t == KC - 1))
                nc.tensor.matmul(csD_t[0:KC, :], sel[:, t * KC:(t + 1) * KC],
                                 swt[:], start=(t == 0), stop=(t == KC - 1))
                engB.dma_start(pt_dram[t * P:(t + 1) * P, :], ptc[:])
            cs_sbC = colp.tile([KC, 512], F16, tag="cs_sbC")
            cs_sbD = colp.tile([KC, 512], F16, tag="cs_sbD")
            nc.scalar.activation(cs_sbC[:], csC_t[0:KC, :], ACT.Copy)
            nc.scalar.activation(cs_sbD[:], csD_t[0:KC, :], ACT.Copy)

            # vch [i-chunk, k in M_c] f16 resident (slotB), fp8 copy in halves
            vch = bigp.tile([P, KC * 512], F16, tag="B", name="vch")
            vch8a = widep.tile([P, HK * 512], F8, tag="w3", name="vch8a")
            vch8b = widep.tile([P, HK * 512], F8, tag="w4", name="vch8b")
            rhsA = widep.tile([P, HK * 512], F16, tag="w1", name="rhsA")
            rhsB = widep.tile([P, HK * 512], F16, tag="w2", name="rhsB")
            c2cols = colp.tile([P, KC], F32, tag="c2cols")
            c3cols = colp.tile([P, NMT * KC], F32, tag="c3cols")

            lhs3 = None
            rhsA3 = vch8a[:].rearrange("a (r m f) -> a r m f", m=NMT, f=512)
            rhsB3 = vch8b[:].rearrange("a (r m f) -> a r m f", m=NMT, f=512)

            # ============ V pass 2 + GEMM, interleaved per strip group ======
            for g in range(NMT):
                # pass 2 for chunks t = 4r+g: exclusive prefix + carry -> vch
                for r8 in range(NC):
                    t = NMT * r8 + g
                    ptc2 = swp.tile([P, R], F16, tag="ptc2", name=f"ptc2_{t}")
                    nc.scalar.dma_start(ptc2[:], pt_dram[t * P:(t + 1) * P, :])
                    swt2 = swp.tile([P, 512], F16, tag="sw2", name=f"sw2_{t}")
                    nc.vector.tensor_scalar(swt2[:], ptc2[:],
                                            s_col[:, t:t + 1], None, OP.mult)
                    cpsL = ps_tile("pb6")
                    cpsR = ps_tile("pb7")
                    nc.tensor.matmul(cpsL[:], tri128[:], ptc2[:],
                                     start=True, stop=False)
                    nc.tensor.matmul(cpsL[:], tribc[:, t * P:(t + 1) * P],
                                     cs_sbC[:], start=False, stop=True)
                    nc.tensor.matmul(cpsR[:], tri128[:], swt2[:],
                                     start=True, stop=False)
                    nc.tensor.matmul(cpsR[:], tribc[:, t * P:(t + 1) * P],
                                     cs_sbD[:], start=False, stop=True)
                    dsb = swp.tile([P, 512], F32, tag="st32", name=f"dsb{t}")
                    nc.scalar.activation(dsb[:], cpsR[:], ACT.Copy)
                    vs = vch[:, t * 512:(t + 1) * 512]
                    nc.vector.scalar_tensor_tensor(
                        vs, cpsL[:], s_col[:, t:t + 1], dsb[:],
                        op0=OP.mult, op1=OP.subtract)
                    v8 = (vch8a if t < HK else vch8b)[
                        :, (t % HK) * 512:(t % HK + 1) * 512]
                    nc.scalar.activation(v8, vs, ACT.Copy)

                # GEMM over contraction strip g: Q_g[jt] = P8[., jt]^T V8
                pnf8_sb = bigp.tile([P, NC * B], F8, tag="C",
                                    name=f"pnf8_sb{g}")
                for r in range(NC):
                    eng = nc.sync if r % 2 == 0 else nc.scalar
                    eng.dma_start(pnf8_sb[:, r * B:(r + 1) * B],
                                  pnf8[g][r * P:(r + 1) * P, :])
                lhs3 = pnf8_sb[:].rearrange("a (r f) -> a r f", f=B)
                if g == 0:
                    # rhs_all = P[:, M_c] from the AllToAll (C2/C3 partner)
                    for t in range(KC):
                        eng = nc.sync if t % 2 == 0 else nc.scalar
                        dst = (rhsA if t < HK else rhsB)[
                            :, (t % HK) * 512:(t % HK + 1) * 512]
                        eng.dma_start(dst, a2a_out[t * P:(t + 1) * P, :])
                for jt in range(KC):
                    qps = ps_tile(f"pb{jt % 6}")
                    for q in range(NMT):
                        rhs3 = rhsA3 if q < 2 else rhsB3
                        ql = q % 2
                        nc.tensor.matmul(
                            qps[:],
                            lhs3[:, 2 * q:2 * q + 2, jt * P:(jt + 1) * P],
                            rhs3[:, 2 * ql:2 * ql + 2, g, :],
                            start=(q == 0), stop=(q == NMT - 1),
                            perf_mode=DR)
                    rhs_j = (rhsA if jt < HK else rhsB)[
                        :, (jt % HK) * 512:(jt % HK + 1) * 512]
                    junk3 = swp.tile([P, 512], F16, tag="jk",
                                     name=f"j3_{g}_{jt}")
                    nc.vector.scalar_tensor_tensor(
                        junk3[:], qps[:], 1.0, rhs_j,
                        op0=OP.mult, op1=OP.mult,
                        accum_out=c3cols[:, g * KC + jt:g * KC + jt + 1])

            # ============ C2 (deferred): <P[:,M_c], V> ======================
            for t in range(KC):
                rhs_j = (rhsA if t < HK else rhsB)[
                    :, (t % HK) * 512:(t % HK + 1) * 512]
                junk2 = swp.tile([P, 512], F16, tag="jk", name=f"j2_{t}")
                nc.vector.scalar_tensor_tensor(
                    junk2[:], rhs_j, 1.0, vch[:, t * 512:(t + 1) * 512],
                    op0=OP.mult, op1=OP.mult,
                    accum_out=c2cols[:, t:t + 1])

            # ============ final reduction ==================================
            c1r = colp.tile([P, NMT], F32, tag="c1r")
            nc.vector.tensor_tensor(c1r[:], c1cols[:], invS[:], OP.mult)
            c1v = colp.tile([P, 1], F32, tag="c1v")
            nc.vector.reduce_sum(c1v[:], c1r[:], axis=AX.X)
            c2v = colp.tile([P, 1], F32, tag="c2v")
            c3v = colp.tile([P, 1], F32, tag="c3v")
            nc.vector.reduce_sum(c2v[:], c2cols[:], axis=AX.X)
            nc.vector.reduce_sum(c3v[:], c3cols[:], axis=AX.X)
            tot = colp.tile([P, 1], F32, tag="tot")
            nc.vector.tensor_scalar(tot[:], c2v[:], 0.5, None, OP.mult)
            nc.vector.tensor_tensor(tot[:], tot[:], c1v[:], OP.add)
            nc.vector.tensor_scalar(c3v[:], c3v[:], 1.0 / 3.0, None, OP.mult)
            nc.vector.tensor_tensor(tot[:], tot[:], c3v[:], OP.add)

            nc.sync.dma_start(vch_out[:, :], vch[:])
            nc.scalar.dma_start(rhs_out[:, 0:HK * 512], rhsA[:])
            nc.scalar.dma_start(rhs_out[:, HK * 512:KC * 512], rhsB[:])
            dbg = colp.tile([P, 8], F32, tag="dbg")
            nc.vector.tensor_copy(dbg[:, 0:1], c1v[:])
            nc.vector.tensor_copy(dbg[:, 1:2], c2v[:])
            nc.vector.tensor_copy(dbg[:, 2:3], c3v[:])
            nc.vector.tensor_copy(dbg[:, 3:4], invS[:, 0:1])
            nc.vector.tensor_copy(dbg[:, 4:5], yt_cols[:, 0:1])
            nc.vector.tensor_copy(dbg[:, 5:6], rs_cols[:, 0:1])
            nc.vector.tensor_copy(dbg[:, 6:7], S_col[:, 0:1])
            nc.vector.tensor_copy(dbg[:, 7:8], c2cols[:, 0:1])
            nc.sync.dma_start(dbg_out[:, :], dbg[:])

            fin = ps_tile("pb7")
            nc.tensor.matmul(fin[0:1, 0:1], tot[:], ones128f[:],
                             start=True, stop=True)
            lsb = colp.tile([1, 8], F32, tag="lsb")
            nc.vector.memset(lsb[:], 0.0)
            nc.scalar.activation(lsb[:, 0:1], fin[0:1, 0:1], ACT.Copy,
                                 scale=1.0 / float(B))
            nc.sync.dma_start(red_in[:, :], lsb[:])
            nc.gpsimd.collective_compute(
                "AllReduce", OP.add, replica_groups=rg,
                ins=[red_in.ap().opt()], outs=[red_out.ap().opt()])
            nc.sync.dma_start(loss_out[:, :], red_out[0:1, 0:1])

    nc.compile()
    return nc


def make_inputs(features, scores, labels, B, D, NC):
    """Build the per-core input maps from full inputs (sorted by score)."""
    R = B // NC
    P = 128
    NMT = R // P
    KC = B // P
    s0 = np.ascontiguousarray(scores, dtype=np.float32).reshape(B)
    order = np.argsort(s0, kind="stable")
    f = np.ascontiguousarray(np.asarray(features, dtype=np.float32)[order])
    s = s0[order]
    lab = np.asarray(labels).astype(np.float32).reshape(B)[order] * 4000.0
    ftf = np.ascontiguousarray(f.T)
    # device constants
    tri128 = np.triu(np.ones((P, P), dtype=np.float32), 1)  # [p<m]
    tri32 = np.triu(np.ones((KC, KC), dtype=np.float32), 1)
    tribc = np.repeat(tri32[:, :, None], P, axis=2).reshape(KC, KC * P)
    sel = np.zeros((P, KC * KC), dtype=np.float32)
    for u in range(KC):
        sel[:, u * KC + u] = 1.0
    in_maps = []
    for c in range(NC):
        sh = slice(c * R, (c + 1) * R)
        in_maps.append({
            "ftf": ftf.astype(ml_dtypes.bfloat16),
            "ft2": np.ascontiguousarray(2.0 * f[sh].T).astype(ml_dtypes.bfloat16),
            "fsh": np.ascontiguousarray(f[sh]),
            "srow16": s.reshape(1, B).astype(np.float16),
            "lrow16": lab.reshape(1, B).astype(np.float16),
            "scolsf": np.ascontiguousarray(s.reshape(KC, P).T),
            "sc_own": np.ascontiguousarray(s[sh].reshape(NMT, P).T),
            "labBIG": np.ascontiguousarray(lab[sh].reshape(NMT, P).T),
            "tri_in": tri128.astype(np.float16),
            "tribc_in": tribc.astype(np.float16),
            "sel_in": sel.astype(np.float16),
        })
    return in_maps


_cached = {}


def kernel(features, scores, labels):
    B, D = features.shape
    NC = 8
    key = (B, D)
    if key not in _cached:
        _cached[key] = build_program(B=B, D=D, NC=NC)
    nc = _cached[key]
    from concourse.bass_utils import run_bass_kernel_spmd
    in_maps = make_inputs(features, scores, labels, B, D, NC)
    res = run_bass_kernel_spmd(nc, in_maps, core_ids=list(range(NC)))
    out = res.results[0]["loss"]
    return np.float32(out.reshape(())[()])


# revision 21
# speedup vs baseline: 1.2728x; 1.0104x over previous
"""Trainium2 Bass kernel for nn_DirectedODRLoss (retrieval_knn).

Math (B=4096, D=256, k=25, scales (1,2,3)), rows sorted by score on host:
    pen_ij = relu(s_i - s_j) = (s_i - s_j)[j < i]  (strict lower tri), so
        V := pen @ P^T,  V[i,k] = s_i C_k(i) - D_k(i)
    with C_k(i) = sum_{j<i} P[k,j], D_k(i) = sum_{j<i} s_j P[k,j] -- exclusive
    prefix sums along sorted columns via per-128-chunk triangular matmuls plus
    rank-1 carry matmuls against prefixed column sums (O(B^2)).

    dist2(i,j) = |f_i|^2 + |f_j|^2 - 2 f_i.f_j ;  y := -dist2 (fp16 strips)
    topk: 3 rounds of DVE max8 + match_replace (top-24), then a plain
        reduce-max of the remainder gives the 25th value tau_i exactly.
    sigma_i = mean sqrt(-y_clamped+eps) over the 25 NN;  rsig = 1/sigma
    mutual knn + direction are folded into the exp argument:
        thr2 = max(tau_i, tau_j, lab4k_i - lab4k_j - 2000)   (labels *4000)
        e3 = y rsig_i rsig_j + 1000*min(y - thr2, 0)  ->  exp underflows to 0
    Wn = exp(e3),  S_i = sum Wn + 1,  P = Wn/S + diag(1/S) (diag written 2/S)
    loss = (1/B)(C1 + C2/2 + C3/3)
        C1 = <P, pen>,  C2 = <P, V>,  C3 = <P@P, V> = <P, Q>,  Q = P^T V

Distribution (8 cores, row shard M_c = rows [c*512, (c+1)*512)):
    - one tiny f16 stats AllGather (tau ++ rsig) right after topk.
    - P shard built row-major (fp16 pn_dram + fp8 pn8_dram); fp8 strips
      all-gathered in two halves (strips 01 after W strip 1, strips 23 at
      the end) -> full row-major P in fp8 for the GEMM.
    - AllToAll of pn blocks delivers a2a_out = P[:, M_c]; its 4 column
      tiles are XBAR-transposed into rhsT = (P[:, M_c])^T for the C3
      contraction partner; C2 streams a2a_out chunks at the end.
    - V[:, M_c] from own P^T chunks (XBAR-transposed out of pn_dram on the
      fly); vch stays SBUF-resident; vch8 (fp8) is the GEMM's stationary
      operand, reused across 8 moving tiles per weight load.
    - C3 GEMM: Q^T-free accumulation: for strip g, kt, q-pair:
      psum[kt,jb] += vch8-pair^T @ pnf8-pair (fp8e4 DoubleRow, K=256/instr),
      then <psum, rhsT tile> accumulated via tensor_tensor_reduce.
"""

import ml_dtypes
import numpy as np

import concourse.bacc as bacc
import concourse.bass as bass
import concourse.mybir as mybir
import concourse.tile as tile

F32 = mybir.dt.float32
F16 = mybir.dt.float16
BF16 = mybir.dt.bfloat16
F8 = mybir.dt.float8e4
AX = mybir.AxisListType
OP = mybir.AluOpType
ACT = mybir.ActivationFunctionType
DR = mybir.MatmulPerfMode.DoubleRow

EPS = 1e-8
KNN = 25
NEG_INF = -60000.0


def build_program(B=4096, D=256, NC=8):
    P = 128
    R = B // NC            # rows per core (512)
    NMT = R // P           # row strips per core (4)
    KC = B // P            # 128-row chunks of B (32)
    GK = D // P            # contraction chunks for the Gram GEMM (2)
    NW = B // 512          # 512-wide column chunks (8)
    HK = KC // 2           # chunks per vch8 half (16)

    nc = bacc.Bacc("TRN2", target_bir_lowering=False, debug=False,
                   num_devices=NC)

    # ---- I/O ----------------------------------------------------------------
    ftf = nc.dram_tensor("ftf", [D, B], BF16, kind="ExternalInput")    # F^T full
    ft2 = nc.dram_tensor("ft2", [D, R], BF16, kind="ExternalInput")    # 2 F_sh^T
    fsh = nc.dram_tensor("fsh", [R, D], F32, kind="ExternalInput")     # F shard
    srow16 = nc.dram_tensor("srow16", [1, B], F16, kind="ExternalInput")
    lrow16 = nc.dram_tensor("lrow16", [1, B], F16, kind="ExternalInput")  # lab*4000
    scolsf = nc.dram_tensor("scolsf", [P, KC], F32, kind="ExternalInput")
    sc_own = nc.dram_tensor("sc_own", [P, NMT], F32, kind="ExternalInput")
    labBIG = nc.dram_tensor("labBIG", [P, NMT], F32, kind="ExternalInput")  # lab*4000
    tri_in = nc.dram_tensor("tri_in", [P, P], F16, kind="ExternalInput")
    tribc_in = nc.dram_tensor("tribc_in", [KC, KC * P], F16,
                              kind="ExternalInput")
    sel_in = nc.dram_tensor("sel_in", [P, KC * KC], F16, kind="ExternalInput")
    loss_out = nc.dram_tensor("loss", [1, 1], F32, kind="ExternalOutput")
    dbg_out = nc.dram_tensor("dbg", [P, 8], F32, kind="ExternalOutput")

    # ---- internal DRAM ------------------------------------------------------
    pn_dram = nc.dram_tensor("pn_dram", [R, B], F16)       # P shard row-major
    pn8_dram = nc.dram_tensor("pn8_dram", [R, B], F8)      # fp8 copy
    pnf8_01 = nc.dram_tensor("pnf8_01", [NC * 2 * P, B], F8, addr_space="Shared")
    pnf8_23 = nc.dram_tensor("pnf8_23", [NC * 2 * P, B], F8, addr_space="Shared")
    pt_dram = nc.dram_tensor("pt_dram", [B, R], F16)       # P_sh^T (own)
    a2a_in = nc.dram_tensor("a2a_in", [B, R], F16)
    a2a_out = nc.dram_tensor("a2a_out", [B, R], F16)       # P[:, M_c]
    st_in = nc.dram_tensor("st_in", [1, 2 * R], F16)
    st_out = nc.dram_tensor("st_out", [NC, 2 * R], F16, addr_space="Shared")
    invs_dram = nc.dram_tensor("invs_dram", [P, NMT], F32)
    red_in = nc.dram_tensor("red_in", [1, 8], F32)
    red_out = nc.dram_tensor("red_out", [1, 8], F32, addr_space="Shared")

    rg = [list(range(NC))]

    with tile.TileContext(nc) as tc:
        with (
            tc.tile_pool(name="const", bufs=1) as constp,
            tc.tile_pool(name="wide", bufs=1) as widep,
            tc.tile_pool(name="big", bufs=1) as bigp,
            tc.tile_pool(name="cols", bufs=1) as colp,
            tc.tile_pool(name="work", bufs=1) as workp,
            tc.tile_pool(name="sw", bufs=2) as swp,
            tc.tile_pool(name="psum", bufs=1, space="PSUM") as psump,
        ):
            def ps_tile(tag, shape=None, dtype=F32):
                return psump.tile(shape or [P, 512], dtype, tag=tag, name=tag)

            # ============ consts ============================================
            tri128 = constp.tile([P, P], F16, tag="tri128")  # [p<m] strict
            nc.sync.dma_start(tri128[:], tri_in[:, :])
            tribc = constp.tile([KC, KC * P], F16, tag="tribc")
            nc.sync.dma_start(tribc[:], tribc_in[:, :])
            sel = constp.tile([P, KC * KC], F16, tag="sel")
            nc.sync.dma_start(sel[:], sel_in[:, :])
            ones128f = constp.tile([P, 1], F32, tag="ones128f")
            nc.vector.memset(ones128f[:], 1.0)
            ones1f = constp.tile([1, P], F32, tag="ones1f")
            nc.vector.memset(ones1f[:], 1.0)
            ones1h = constp.tile([1, P], F16, tag="ones1h")
            nc.vector.memset(ones1h[:], 1.0)
            eps_c = constp.tile([P, 1], F32, tag="eps_c")
            nc.vector.memset(eps_c[:], EPS)

            # ============ input loads =======================================
            ft2_sb = constp.tile([P, GK * R], BF16, tag="ft2_sb")
            for g in range(GK):
                nc.sync.dma_start(ft2_sb[:, g * R:(g + 1) * R],
                                  ft2[g * P:(g + 1) * P, :])
            # slotA: ftf (Gram) -> pnf8_sb (odd GEMM strips)
            ftf_sb = bigp.tile([P, GK * B], BF16, tag="A", name="ftf_sb")
            for g in range(GK):
                nc.sync.dma_start(ftf_sb[:, g * B:(g + 1) * B],
                                  ftf[g * P:(g + 1) * P, :])
            s_col = colp.tile([P, KC], F32, tag="s_col")
            nc.sync.dma_start(s_col[:], scolsf[:, :])
            s_c = colp.tile([P, NMT], F32, tag="s_c")
            nc.sync.dma_start(s_c[:], sc_own[:, :])
            lab4k_c = colp.tile([P, NMT], F32, tag="lab4k_c")
            nc.sync.dma_start(lab4k_c[:], labBIG[:, :])
            labm2k = colp.tile([P, NMT], F32, tag="labm2k")
            nc.vector.tensor_scalar(labm2k[:], lab4k_c[:], 2000.0, None,
                                    OP.subtract)
            # wide broadcast rows (tags w1..w4; w1/w2 reused by rhsT after W)
            s_b16 = widep.tile([P, B], F16, tag="w1", name="s_b16")
            nc.sync.dma_start(s_b16[:], bass.AP(srow16, 0, [[0, P], [1, B]]))
            lab4k_b16 = widep.tile([P, B], F16, tag="w2", name="lab4k_b16")
            nc.sync.dma_start(lab4k_b16[:], bass.AP(lrow16, 0, [[0, P], [1, B]]))
            yt_b16 = widep.tile([P, B], F16, tag="w3", name="yt_b16")
            rs_b16 = widep.tile([P, B], F16, tag="w4", name="rs_b16")

            # own |f_i|^2 in col layout (bias for the y copy-out)
            sqcs = colp.tile([P, NMT], F32, tag="sqcs")
            for q in range(NMT):
                ftile = swp.tile([P, 512], F32, tag="sqq", name=f"fsh{q}")
                nc.sync.dma_start(ftile[:, 0:D], fsh[q * P:(q + 1) * P, :])
                scr = swp.tile([P, 512], F32, tag="st32", name=f"fsq{q}")
                nc.scalar.activation(scr[:, 0:D], ftile[:, 0:D], ACT.Square,
                                     accum_out=sqcs[:, q:q + 1])
            sqcs_neg = colp.tile([P, NMT], F32, tag="sqcs_neg")
            nc.vector.tensor_scalar(sqcs_neg[:], sqcs[:], -1.0, None, OP.mult)

            # |f_j|^2 row strip (partition-reduce by matmul, then broadcast)
            sqb16 = workp.tile([P, B], F16, tag="t4", name="sqb16")
            for o in range(NW):
                pso = ps_tile(f"pb{o}")
                for g in range(GK):
                    sqq = swp.tile([P, 512], F32, tag="sqq",
                                   name=f"sqq{o}_{g}")
                    nc.scalar.activation(
                        sqq[:], ftf_sb[:, g * B + o * 512: g * B + (o + 1) * 512],
                        ACT.Square)
                    nc.tensor.matmul(pso[0:1, :], ones128f[:], sqq[:],
                                     start=(g == 0), stop=(g == GK - 1))
                sqr = swp.tile([P, 512], F32, tag="st32", name=f"sqr{o}")
                nc.scalar.activation(sqr[0:1, :], pso[0:1, :], ACT.Copy)
                nc.tensor.matmul(pso[:], ones1f[:], sqr[0:1, :],
                                 start=True, stop=True)
                nc.vector.tensor_copy(sqb16[:, o * 512:(o + 1) * 512], pso[:])

            # ============ Gram -> y (fp16 strips) + topk ====================
            # slotB: y strips (Wn in place) -> vch after
            y_all = bigp.tile([P, NMT * B], F16, tag="B", name="y_all")
            vals = colp.tile([P, NMT * 32], F16, tag="vals")
            yt_cols = colp.tile([P, NMT], F32, tag="yt_cols")
            rs_cols = colp.tile([P, NMT], F32, tag="rs_cols")
            ssum = colp.tile([P, NMT], F32, tag="ssum")
            stf16 = colp.tile([P, 2 * NMT], F16, tag="stf16")
            for mt in range(NMT):
                ys = y_all[:, mt * B:(mt + 1) * B]
                for o in range(NW):
                    pso = ps_tile(f"pb{o}")
                    for g in range(GK):
                        nc.tensor.matmul(
                            pso[:],
                            ft2_sb[:, g * R + mt * P: g * R + (mt + 1) * P],
                            ftf_sb[:, g * B + o * 512: g * B + (o + 1) * 512],
                            start=(g == 0), stop=(g == GK - 1))
                    nc.vector.scalar_tensor_tensor(
                        ys[:, o * 512:(o + 1) * 512], pso[:],
                        sqcs_neg[:, mt:mt + 1],
                        sqb16[:, o * 512:(o + 1) * 512],
                        op0=OP.add, op1=OP.subtract)
                # top-25: 3 rounds max8 + match_replace, then plain reduce-max
                sa = workp.tile([P, B], F16, tag="t1", name=f"sa{mt}")
                sb = workp.tile([P, B], F16, tag="t2", name=f"sb{mt}")
                v = vals[:, mt * 32:(mt + 1) * 32]
                nc.vector.max(out=v[:, 0:8], in_=ys)
                nc.vector.match_replace(out=sb[:], in_to_replace=v[:, 0:8],
                                        in_values=ys, imm_value=NEG_INF)
                nc.vector.max(out=v[:, 8:16], in_=sb[:])
                nc.vector.match_replace(out=sa[:], in_to_replace=v[:, 8:16],
                                        in_values=sb[:], imm_value=NEG_INF)
                nc.vector.max(out=v[:, 16:24], in_=sa[:])
                nc.vector.match_replace(out=sb[:], in_to_replace=v[:, 16:24],
                                        in_values=sa[:], imm_value=NEG_INF)
                nc.vector.reduce_max(v[:, 24:25], sb[:], axis=AX.X)
                nc.vector.tensor_copy(yt_cols[:, mt:mt + 1], v[:, 24:25])
                # sigma_i = mean sqrt(max(d,0)+eps) over 25 NN;  d = -y
                c25 = swp.tile([P, KNN], F32, tag="c25")
                nc.vector.tensor_scalar(c25[:], v[:, 0:KNN], 0.0, None, OP.min)
                s25 = swp.tile([P, KNN], F32, tag="s25")
                nc.scalar.activation(s25[:], c25[:], ACT.Sqrt,
                                     bias=eps_c[:, 0:1], scale=-1.0,
                                     accum_out=ssum[:, mt:mt + 1])
                nc.vector.reciprocal(rs_cols[:, mt:mt + 1], ssum[:, mt:mt + 1])
                nc.vector.tensor_scalar(rs_cols[:, mt:mt + 1],
                                        rs_cols[:, mt:mt + 1], float(KNN),
                                        None, OP.mult)
                nc.vector.tensor_copy(stf16[:, mt:mt + 1],
                                      yt_cols[:, mt:mt + 1])
                nc.vector.tensor_copy(stf16[:, NMT + mt:NMT + mt + 1],
                                      rs_cols[:, mt:mt + 1])

            # single stats AllGather: [tau(R) ++ rsig(R)] f16, shard order
            nc.sync.dma_start(bass.AP(st_in, 0, [[1, P], [P, NMT]]),
                              stf16[:, 0:NMT])
            nc.sync.dma_start(bass.AP(st_in, R, [[1, P], [P, NMT]]),
                              stf16[:, NMT:2 * NMT])
            nc.gpsimd.collective_compute(
                "AllGather", OP.bypass, replica_groups=rg,
                ins=[st_in.ap().opt()], outs=[st_out.ap().opt()])
            for r in range(NC):
                eng = nc.sync if r % 2 == 0 else nc.scalar
                eng.dma_start(yt_b16[:, r * R:(r + 1) * R],
                              bass.AP(st_out, r * 2 * R, [[0, P], [1, R]]))
                eng.dma_start(rs_b16[:, r * R:(r + 1) * R],
                              bass.AP(st_out, r * 2 * R + R, [[0, P], [1, R]]))

            # ============ stage W: Wn, S, C1, P strips + collectives ========
            S_col = colp.tile([P, NMT], F32, tag="S_col")
            c1cols = colp.tile([P, NMT], F32, tag="c1cols")
            invS = colp.tile([P, NMT], F32, tag="invS")
            Scol = colp.tile([P, NMT], F32, tag="Scol")
            rank = nc.gpsimd.partition_id()
            pn_flat = pn_dram.ap().rearrange("a b -> () (a b)")
            pn8_flat = pn8_dram.ap().rearrange("a b -> () (a b)")
            a2a_flat = a2a_in.ap().rearrange("a b -> () (a b)")
            for mt in range(NMT):
                ys = y_all[:, mt * B:(mt + 1) * B]
                # vb = lab4k_i - lab4k_j - 2000  (>=2000 iff direction violated)
                vb = workp.tile([P, B], F16, tag="t1", name=f"vb{mt}")
                nc.vector.tensor_scalar(vb[:], lab4k_b16[:], -1.0,
                                        labm2k[:, mt:mt + 1], OP.mult, OP.add)
                amx = workp.tile([P, B], F16, tag="t2", name=f"amx{mt}")
                nc.vector.tensor_scalar(amx[:], yt_b16[:],
                                        yt_cols[:, mt:mt + 1], None, OP.max)
                thr2 = workp.tile([P, B], F16, tag="t6", name=f"thr2{mt}")
                nc.vector.tensor_tensor(thr2[:], amx[:], vb[:], OP.max)
                dk = workp.tile([P, B], F16, tag="t1", name=f"dk{mt}")
                nc.vector.tensor_tensor(dk[:], ys, thr2[:], OP.subtract)
                dm = workp.tile([P, B], F16, tag="t2", name=f"dm{mt}")
                nc.vector.tensor_scalar(dm[:], dk[:], 0.0, 1000.0,
                                        OP.min, OP.mult)
                # keep exp's argument finite: hw exp of -inf is unsafe
                nc.vector.tensor_scalar(dm[:], dm[:], -30000.0, None, OP.max)
                e1 = workp.tile([P, B], F16, tag="t1", name=f"e1{mt}")
                nc.vector.tensor_scalar(e1[:], ys, rs_cols[:, mt:mt + 1],
                                        None, OP.mult)
                e2 = workp.tile([P, B], F16, tag="t6", name=f"e2{mt}")
                nc.vector.tensor_tensor(e2[:], e1[:], rs_b16[:], OP.mult)
                e3 = workp.tile([P, B], F16, tag="t1", name=f"e3{mt}")
                nc.vector.tensor_tensor(e3[:], e2[:], dm[:], OP.add)
                # Wn = exp(e3) into the y strip, accumulating S on the way
                nc.scalar.activation(ys, e3[:], ACT.Exp,
                                     accum_out=S_col[:, mt:mt + 1])
                # C1 partial: sum Wn pen
                pen = workp.tile([P, B], F16, tag="t6", name=f"pen{mt}")
                nc.scalar.activation(pen[:], s_b16[:], ACT.Relu,
                                     bias=s_c[:, mt:mt + 1], scale=-1.0)
                junk = workp.tile([P, B], F16, tag="t2", name=f"cj{mt}")
                nc.vector.scalar_tensor_tensor(
                    junk[:], ys, 1.0, pen[:], op0=OP.mult, op1=OP.mult,
                    accum_out=c1cols[:, mt:mt + 1])
                nc.vector.tensor_scalar(Scol[:, mt:mt + 1],
                                        S_col[:, mt:mt + 1], 1.0, None, OP.add)
                nc.vector.reciprocal(invS[:, mt:mt + 1], Scol[:, mt:mt + 1])
                # P strip: fp16 (pn_dram) + fp8 (pn8_dram)
                pn16 = workp.tile([P, B], F16, tag="t4", name=f"pn16_{mt}")
                nc.scalar.activation(pn16[:], ys, ACT.Copy,
                                     scale=invS[:, mt:mt + 1])
                pn8 = workp.tile([P, B], F8, tag="t5", name=f"pn8_{mt}")
                nc.vector.tensor_scalar(pn8[:], ys, invS[:, mt:mt + 1], None,
                                        OP.mult)
                nc.gpsimd.dma_start(pn_dram[mt * P:(mt + 1) * P, :], pn16[:])
                nc.gpsimd.dma_start(pn8_dram[mt * P:(mt + 1) * P, :], pn8[:])
                # AllToAll input: block r' = own strip rows, columns M_r'
                for rp in range(NC):
                    nc.gpsimd.dma_start(
                        a2a_in[rp * R + mt * P: rp * R + (mt + 1) * P, :],
                        pn16[:, rp * R:(rp + 1) * R])
                # diagonal: write 2/S (self weight Wn=1 plus eye)
                nc.sync.dma_start(invs_dram[:, mt:mt + 1], invS[:, mt:mt + 1])
                d_rowf = swp.tile([1, P], F32, tag="d_rowf", name=f"drf{mt}")
                nc.sync.dma_start(d_rowf[:],
                                  bass.AP(invs_dram, mt, [[0, 1], [NMT, P]]))
                d2_16 = swp.tile([1, P], F16, tag="d2_16", name=f"d16{mt}")
                nc.vector.tensor_scalar(d2_16[:], d_rowf[:], 2.0, None, OP.mult)
                d2_8 = swp.tile([1, P], F8, tag="d2_8", name=f"d8{mt}")
                nc.vector.tensor_scalar(d2_8[:], d_rowf[:], 2.0, None, OP.mult)
                nc.gpsimd.dma_start(
                    pn_flat[0:1, bass.ds(rank * R + mt * P * (B + 1), P, B + 1)],
                    d2_16[0:1, :])
                nc.gpsimd.dma_start(
                    pn8_flat[0:1, bass.ds(rank * R + mt * P * (B + 1), P, B + 1)],
                    d2_8[0:1, :])
                nc.gpsimd.dma_start(
                    a2a_flat[0:1,
                             bass.ds(rank * R * R + mt * P * (R + 1), P, R + 1)],
                    d2_16[0:1, :])
                if mt == 1:
                    nc.gpsimd.collective_compute(
                        "AllGather", OP.bypass, replica_groups=rg,
                        ins=[pn8_dram[0:2 * P, :].opt()],
                        outs=[pnf8_01.ap().opt()])

            # CC queue: A2A (needs all strips), then AG of strips 23
            nc.gpsimd.collective_compute(
                "AllToAll", OP.bypass, replica_groups=rg,
                ins=[a2a_in.ap().opt()], outs=[a2a_out.ap().opt()])
            nc.gpsimd.collective_compute(
                "AllGather", OP.bypass, replica_groups=rg,
                ins=[pn8_dram[2 * P:4 * P, :].opt()],
                outs=[pnf8_23.ap().opt()])

            # ============ V pass 1: transpose chunks + column sums ==========
            csC_t = ps_tile("pb6")
            csD_t = ps_tile("pb7")
            for t in range(KC):
                engA = nc.sync if t % 2 == 0 else nc.scalar
                engB = nc.scalar if t % 2 == 0 else nc.sync
                ptc = swp.tile([P, R], F16, tag="ptc", name=f"ptc1_{t}")
                engA.dma_start_transpose(ptc[:],
                                         pn_dram[0:R, t * P:(t + 1) * P])
                swt = swp.tile([P, 512], F16, tag="sw1", name=f"sw1_{t}")
                nc.vector.tensor_scalar(swt[:], ptc[:],
                                        s_col[:, t:t + 1], None, OP.mult)
                nc.tensor.matmul(csC_t[0:KC, :], sel[:, t * KC:(t + 1) * KC],
                                 ptc[:], start=(t == 0), stop=(t == KC - 1))
                nc.tensor.matmul(csD_t[0:KC, :], sel[:, t * KC:(t + 1) * KC],
                                 swt[:], start=(t == 0), stop=(t == KC - 1))
                engB.dma_start(pt_dram[t * P:(t + 1) * P, :], ptc[:])
            cs_sbC = colp.tile([KC, 512], F16, tag="cs_sbC")
            cs_sbD = colp.tile([KC, 512], F16, tag="cs_sbD")
            nc.scalar.activation(cs_sbC[:], csC_t[0:KC, :], ACT.Copy)
            nc.scalar.activation(cs_sbD[:], csD_t[0:KC, :], ACT.Copy)
            # rhsT = (P[:, M_c])^T: XBAR transposes of a2a_out columns in
            # [512,128] blocks (issued after pass 1 so they don't block it
            # on the DMA queues while the A2A is still in flight)
            rhsTa = widep.tile([P, 2 * B], F16, tag="w1", name="rhsTa")
            rhsTb = widep.tile([P, 2 * B], F16, tag="w2", name="rhsTb")
            for kt in range(NMT):
                for cb in range(NC):
                    dst = (rhsTa if kt < 2 else rhsTb)[
                        :, (kt % 2) * B + cb * 512:(kt % 2) * B + (cb + 1) * 512]
                    eng = nc.sync if (kt * NC + cb) % 2 == 0 else nc.scalar
                    eng.dma_start_transpose(
                        dst, a2a_out[cb * 512:(cb + 1) * 512,
                                     kt * P:(kt + 1) * P])

            # vch [i-chunk, k in M_c] f16 resident (slotB), fp8 copy in halves
            vch = bigp.tile([P, KC * 512], F16, tag="B", name="vch")
            vch8a = widep.tile([P, HK * 512], F8, tag="w3", name="vch8a")
            vch8b = widep.tile([P, HK * 512], F8, tag="w4", name="vch8b")
            c2cols = colp.tile([P, KC], F32, tag="c2cols")
            c3cols = colp.tile([P, NMT * KC], F32, tag="c3cols")

            vA4 = vch8a[:].rearrange("a (r m f) -> a r m f", m=NMT, f=512)
            vB4 = vch8b[:].rearrange("a (r m f) -> a r m f", m=NMT, f=512)

            # ============ V pass 2 + GEMM, interleaved per strip group ======
            for g in range(NMT):
                # pass 2 for chunks t = 4r+g: exclusive prefix + carry -> vch
                for r8 in range(NC):
                    t = NMT * r8 + g
                    ptc2 = swp.tile([P, R], F16, tag="ptc2", name=f"ptc2_{t}")
                    nc.scalar.dma_start(ptc2[:], pt_dram[t * P:(t + 1) * P, :])
                    swt2 = swp.tile([P, 512], F16, tag="sw2", name=f"sw2_{t}")
                    nc.vector.tensor_scalar(swt2[:], ptc2[:],
                                            s_col[:, t:t + 1], None, OP.mult)
                    cpsL = ps_tile("pb6")
                    cpsR = ps_tile("pb7")
                    nc.tensor.matmul(cpsL[:], tri128[:], ptc2[:],
                                     start=True, stop=False)
                    nc.tensor.matmul(cpsL[:], tribc[:, t * P:(t + 1) * P],
                                     cs_sbC[:], start=False, stop=True)
                    nc.tensor.matmul(cpsR[:], tri128[:], swt2[:],
                                     start=True, stop=False)
                    nc.tensor.matmul(cpsR[:], tribc[:, t * P:(t + 1) * P],
                                     cs_sbD[:], start=False, stop=True)
                    dsb = swp.tile([P, 512], F32, tag="st32",
                                   name=f"dsb{t}")
                    nc.scalar.activation(dsb[:], cpsR[:], ACT.Copy)
                    vs = vch[:, t * 512:(t + 1) * 512]
                    nc.vector.scalar_tensor_tensor(
                        vs, cpsL[:], s_col[:, t:t + 1], dsb[:],
                        op0=OP.mult, op1=OP.subtract)
                    v8 = (vch8a if t < HK else vch8b)[
                        :, (t % HK) * 512:(t % HK + 1) * 512]
                    nc.vector.tensor_copy(v8, vs)

                # GEMM over contraction strip g (vch8 stationary, reused x8)
                pnf8_sb = bigp.tile([P, NC * B], F8, tag="C",
                                    name=f"pnf8_sb{g}")
                src = pnf8_01 if g < 2 else pnf8_23
                for r in range(NC):
                    eng = nc.sync if r % 2 == 0 else nc.scalar
                    eng.dma_start(
                        pnf8_sb[:, r * B:(r + 1) * B],
                        src[r * 2 * P + (g % 2) * P:
                            r * 2 * P + (g % 2 + 1) * P, :])
                pn3 = pnf8_sb[:].rearrange("a (r f) -> a r f", f=B)
                for kt in range(NMT):
                    for jbs, base in (((0, 1, 2, 3, 4, 5), 0), ((6, 7), 0)):
                        pss = {jb: ps_tile(f"pb{jb - jbs[0] + base}")
                               for jb in jbs}
                        for q in range(NMT):
                            v4 = vA4 if q < 2 else vB4
                            ql = q % 2
                            lhsT = v4[:, 2 * ql:2 * ql + 2, g,
                                      kt * P:(kt + 1) * P]
                            for jb in jbs:
                                nc.tensor.matmul(
                                    pss[jb][:], lhsT,
                                    pn3[:, 2 * q:2 * q + 2,
                                        jb * 512:(jb + 1) * 512],
                                    start=(q == 0), stop=(q == NMT - 1),
                                    perf_mode=DR)
                        for jb in jbs:
                            junk3 = swp.tile([P, 512], F16, tag="jk",
                                             name=f"j3_{g}_{kt}_{jb}")
                            rt = (rhsTa if kt < 2 else rhsTb)[
                                :, (kt % 2) * B + jb * 512:
                                   (kt % 2) * B + (jb + 1) * 512]
                            ci = g * KC + kt * NC + jb
                            nc.vector.scalar_tensor_tensor(
                                junk3[:], pss[jb][:], 1.0, rt,
                                op0=OP.mult, op1=OP.mult,
                                accum_out=c3cols[:, ci:ci + 1])

            # ============ C2 (deferred): <P[:,M_c], V>, streamed ============
            for t in range(KC):
                rchunk = swp.tile([P, 512], F16, tag="rchunk", name=f"rc{t}")
                eng = nc.sync if t % 2 == 0 else nc.scalar
                eng.dma_start(rchunk[:], a2a_out[t * P:(t + 1) * P, :])
                junk2 = swp.tile([P, 512], F16, tag="jk", name=f"j2_{t}")
                nc.vector.scalar_tensor_tensor(
                    junk2[:], rchunk[:], 1.0,
                    vch[:, t * 512:(t + 1) * 512],
                    op0=OP.mult, op1=OP.mult,
                    accum_out=c2cols[:, t:t + 1])

            # ============ final reduction ==================================
            c1r = colp.tile([P, NMT], F32, tag="c1r")
            nc.vector.tensor_tensor(c1r[:], c1cols[:], invS[:], OP.mult)
            c1v = colp.tile([P, 1], F32, tag="c1v")
            nc.vector.reduce_sum(c1v[:], c1r[:], axis=AX.X)
            c2v = colp.tile([P, 1], F32, tag="c2v")
            c3v = colp.tile([P, 1], F32, tag="c3v")
            nc.vector.reduce_sum(c2v[:], c2cols[:], axis=AX.X)
            nc.vector.reduce_sum(c3v[:], c3cols[:], axis=AX.X)
            tot = colp.tile([P, 1], F32, tag="tot")
            nc.vector.tensor_scalar(tot[:], c2v[:], 0.5, None, OP.mult)
            nc.vector.tensor_tensor(tot[:], tot[:], c1v[:], OP.add)
            nc.vector.tensor_scalar(c3v[:], c3v[:], 1.0 / 3.0, None, OP.mult)
            nc.vector.tensor_tensor(tot[:], tot[:], c3v[:], OP.add)

            dbg = colp.tile([P, 8], F32, tag="dbg")
            nc.vector.tensor_copy(dbg[:, 0:1], c1v[:])
            nc.vector.tensor_copy(dbg[:, 1:2], c2v[:])
            nc.vector.tensor_copy(dbg[:, 2:3], c3v[:])
            nc.vector.tensor_copy(dbg[:, 3:4], invS[:, 0:1])
            nc.vector.tensor_copy(dbg[:, 4:5], yt_cols[:, 0:1])
            nc.vector.tensor_copy(dbg[:, 5:6], rs_cols[:, 0:1])
            nc.vector.tensor_copy(dbg[:, 6:7], S_col[:, 0:1])
            nc.vector.tensor_copy(dbg[:, 7:8], c2cols[:, 0:1])
            nc.sync.dma_start(dbg_out[:, :], dbg[:])

            fin = ps_tile("pb7")
            nc.tensor.matmul(fin[0:1, 0:1], tot[:], ones128f[:],
                             start=True, stop=True)
            lsb = colp.tile([1, 8], F32, tag="lsb")
            nc.vector.memset(lsb[:], 0.0)
            nc.scalar.activation(lsb[:, 0:1], fin[0:1, 0:1], ACT.Copy,
                                 scale=1.0 / float(B))
            nc.sync.dma_start(red_in[:, :], lsb[:])
            nc.gpsimd.collective_compute(
                "AllReduce", OP.add, replica_groups=rg,
                ins=[red_in.ap().opt()], outs=[red_out.ap().opt()])
            nc.sync.dma_start(loss_out[:, :], red_out[0:1, 0:1])

    nc.compile()
    return nc


def make_inputs(features, scores, labels, B, D, NC):
    """Build the per-core input maps from full inputs (sorted by score)."""
    R = B // NC
    P = 128
    NMT = R // P
    KC = B // P
    s0 = np.ascontiguousarray(scores, dtype=np.float32).reshape(B)
    order = np.argsort(s0, kind="stable")
    f = np.ascontiguousarray(np.asarray(features, dtype=np.float32)[order])
    s = s0[order]
    lab = np.asarray(labels).astype(np.float32).reshape(B)[order] * 4000.0
    ftf = np.ascontiguousarray(f.T)
    tri128 = np.triu(np.ones((P, P), dtype=np.float32), 1)  # [p<m]
    tri32 = np.triu(np.ones((KC, KC), dtype=np.float32), 1)
    tribc = np.repeat(tri32[:, :, None], P, axis=2).reshape(KC, KC * P)
    sel = np.zeros((P, KC * KC), dtype=np.float32)
    for u in range(KC):
        sel[:, u * KC + u] = 1.0
    in_maps = []
    for c in range(NC):
        sh = slice(c * R, (c + 1) * R)
        in_maps.append({
            "ftf": ftf.astype(ml_dtypes.bfloat16),
            "ft2": np.ascontiguousarray(2.0 * f[sh].T).astype(ml_dtypes.bfloat16),
            "fsh": np.ascontiguousarray(f[sh]),
            "srow16": s.reshape(1, B).astype(np.float16),
            "lrow16": lab.reshape(1, B).astype(np.float16),
            "scolsf": np.ascontiguousarray(s.reshape(KC, P).T),
            "sc_own": np.ascontiguousarray(s[sh].reshape(NMT, P).T),
            "labBIG": np.ascontiguousarray(lab[sh].reshape(NMT, P).T),
            "tri_in": tri128.astype(np.float16),
            "tribc_in": tribc.astype(np.float16),
            "sel_in": sel.astype(np.float16),
        })
    return in_maps


_cached = {}


def kernel(features, scores, labels):
    B, D = features.shape
    NC = 8
    key = (B, D)
    if key not in _cached:
        _cached[key] = build_program(B=B, D=D, NC=NC)
    nc = _cached[key]
    from concourse.bass_utils import run_bass_kernel_spmd
    in_maps = make_inputs(features, scores, labels, B, D, NC)
    res = run_bass_kernel_spmd(nc, in_maps, core_ids=list(range(NC)))
    out = res.results[0]["loss"]
    return np.float32(out.reshape(())[()])
